# revision 32
# baseline (speedup 1.0000x reference)
"""Trainium2 Bass kernel for a dense transformer block (pre-LN attn + MLP).

B=4, T=2048, D=768, H=12 (DH=64), DFF=3072, fp32.

Sharding: 8 cores = 4 batches x 2 roles. Each core processes one batch and
owns 1024 query tokens (two 512-blocks, paired {0,3}/{1,2} for causal load
balance). K/V are computed for the full 2048 tokens on both cores of a batch
(cheap), so there are NO collectives.

SPMD uniformity: all 8 cores run ONE identical NEFF. Causal structure is
carried in DATA, not code:
  - host permutes each batch's token axis to [own0, own1, otherA, otherB]
  - q-slot0 attends s-chunks {0..3, 8..11}; q-slot1 attends s-chunks {0..15}
  - per-(slot,chunk) exp scale/bias inputs select live / dead (zero) chunks
  - 4 canonical triangular masks handle the self-diagonal 512-blocks

Everything on-chip runs in a transposed layout (features on partitions,
tokens on the free axis) so no on-chip transposes are needed; all weight /
input transposes happen on the host in numpy. Matmuls run as float32r
(full PE speed, ~bf16x2 precision). LayerNorm statistics are computed with
ones-column matmuls; per-token stats are broadcast across partitions with
K=1 outer-product matmuls. Softmax denominators come for free from a ones
column appended to V (65-row PV matmul); the divide is folded in after PV.
"""

import sys

sys.path.insert(0, "/opt/trn_rl_repo")

from contextlib import ExitStack

import numpy as np

import concourse.bass as bass
import concourse.mybir as mybir
import concourse.tile as tile
from concourse import bacc
from concourse.bass_utils import run_bass_kernel_spmd

F32 = mybir.dt.float32
F32R = mybir.dt.float32r
AF = mybir.ActivationFunctionType
BF16 = mybir.dt.bfloat16
F8 = mybir.dt.float8e4
DR = mybir.MatmulPerfMode.DoubleRow
ALU = mybir.AluOpType

# fp8 pre-scales (host folds these into the weights; kernel divides out)
S_W1 = 16.0     # W1 stored as 16*W1
S_H1 = 8.0      # h1 stored as 8*relu(.)
S_W2 = 16.0     # W2 stored as 16*W2
C_MLP = 1.0 / (S_H1 * S_W2)          # o2 -> true h@W2.T scale
S_YDIV = 1.0    # PV sums pre-scale before the divide (1 = none)
C_WO = 1.0      # wo-out descale folded into the residual add

H, D, DFF = 12, 768, 3072
DH = 64
B, T = 4, 2048
EPS = 1e-5
P = 128
NC = D // P          # 6 feature chunks
NF = DFF // P        # 24 ff tiles
TB = 512             # token block
NTB = T // TB        # 4 blocks
SLOT_CHUNKS = [[0, 1, 2, 3, 8, 9, 10, 11], list(range(16))]
# role -> permuted block order [own0, own1, restA, restB] (original block ids)
ROLE_ORDER = [[0, 3, 1, 2], [1, 2, 0, 3]]
DEAD = -30000.0      # exp(DEAD) == 0 in fp32

_cached = {}
PHASE_MARKS = []


def _mark(nc, name):
    PHASE_MARKS.append((name, nc.next_id()))


def _build_nc(use_be1=False):
    nc = bacc.Bacc("TRN2", target_bir_lowering=False, debug=False,
                   enable_asserts=False, num_devices=8)

    def din(name, shape, dt=F32R):
        return nc.dram_tensor(name, shape, dt, kind="ExternalInput").ap()

    xt_d = din("xt", [D, T])                 # X[b].T, token-permuted
    wqt_d = din("wqt", [D, D], BF16)         # g1-folded w_q as [c, m]
    wkt_d = din("wkt", [D, D], BF16)
    wvt_d = din("wvt", [D, D], BF16)
    wo_d = din("wo", [D, D], BF16)           # natural [m, c]
    w1t_d = din("w1t", [NF // 4, D, 2, 4 * P], F8)  # [fgrp, c, (hi/lo), f]
    w2t_d = din("w2t", [DFF, 2, D], F8)      # 16*W2.T  [f, (hi/lo), c]
    ones512_d = din("ones512", [1, TB])      # ones row for b2 outer
    b2s_d = din("b2s", [1, D])               # 128*b2 as a row
    onesr_d = din("onesr", [1, P])           # outer-product lhsT
    onesc_d = din("onesc", [P, 1])           # column-sum lhsT
    masks_d = din("masks", [4, P, 2 * TB], BF16)  # tri masks x2 halves
    scalein_d = din("scalein", [P, 24], F32) # exp scale per (slot,chunk)
    biasin_d = din("biasin", [P, 24], F32)   # exp bias per (slot,chunk)
    # LN1 is folded into the QKV path: -colsum(g1-folded w) rows for the
    # rank-1 mean correction (and optional be1-fold rows)
    nwqs_d = din("nwqs", [1, D])
    nwks_d = din("nwks", [1, D])
    nwvs_d = din("nwvs", [1, D])
    if use_be1:
        wqbe_d = din("wqbe", [1, D])
        wkbe_d = din("wkbe", [1, D])
        wvbe_d = din("wvbe", [1, D])
    g2_d = din("g2v", [D], F32)
    be2_d = din("be2v", [D], F32)
    g2r_d = din("g2r", [1, D])
    b1_d = din("b1v", [DFF], F32)

    outt_d = nc.dram_tensor("outt", [D, 1024], F32, kind="ExternalOutput").ap()

    xt_r = xt_d.rearrange("(j p) t -> p j t", p=P)

    with tile.TileContext(nc) as tc, ExitStack() as ctx, \
         nc.allow_low_precision(reason="fp32r/bf16 intermediates are intended"):
        consts = ctx.enter_context(tc.tile_pool(name="consts", bufs=1))
        ps = ctx.enter_context(tc.tile_pool(name="ps", bufs=1, space="PSUM"))
        rows = ctx.enter_context(tc.tile_pool(name="rows", bufs=1))
        work = ctx.enter_context(tc.tile_pool(name="work", bufs=2))

        onesr_sb = consts.tile([1, P], F32R, tag="onesr")
        onesc_sb = consts.tile([P, 1], F32R, tag="onesc")
        scale_sb = consts.tile([P, 24], F32, tag="scalein")
        bias_sb = consts.tile([P, 24], F32, tag="biasin")
        g2_sb = consts.tile([P, NC], F32, tag="g2")
        be2_sb = consts.tile([P, NC], F32, tag="be2")
        g2r_sb = consts.tile([1, D], F32R, tag="g2r")
        b1_sb = consts.tile([P, NF], F32, tag="b1")
        ones512_sb = consts.tile([1, TB], F32R, tag="ones512")
        b2s_sb = consts.tile([1, D], F32R, tag="b2s")
        nwqs_sb = consts.tile([1, D], F32R, tag="nwqs")
        nwks_sb = consts.tile([1, D], F32R, tag="nwks")
        nwvs_sb = consts.tile([1, D], F32R, tag="nwvs")
        if use_be1:
            wqbe_sb = consts.tile([1, D], F32R, tag="wqbe")
            wkbe_sb = consts.tile([1, D], F32R, tag="wkbe")
            wvbe_sb = consts.tile([1, D], F32R, tag="wvbe")

        def _early_const_dmas():
            nc.sync.dma_start(onesc_sb[:], onesc_d)
            nc.sync.dma_start(onesr_sb[:], onesr_d)
            nc.sync.dma_start(nwqs_sb[:], nwqs_d)
            nc.sync.dma_start(nwks_sb[:], nwks_d)
            nc.sync.dma_start(nwvs_sb[:], nwvs_d)
            if use_be1:
                nc.sync.dma_start(wqbe_sb[:], wqbe_d)
                nc.sync.dma_start(wkbe_sb[:], wkbe_d)
                nc.sync.dma_start(wvbe_sb[:], wvbe_d)
                nc.sync.dma_start(ones512_sb[:], ones512_d)

        def _late_const_dmas():
            nc.sync.dma_start(scale_sb[:], scalein_d)
            nc.sync.dma_start(bias_sb[:], biasin_d)
            nc.sync.dma_start(g2r_sb[:], g2r_d)
            for sb, d in ((g2_sb, g2_d), (be2_sb, be2_d)):
                nc.sync.dma_start(sb[:], d.rearrange("(j p) -> p j", p=P))
            nc.sync.dma_start(b1_sb[:], b1_d.rearrange("(j p) -> p j", p=P))
            if not use_be1:
                nc.sync.dma_start(ones512_sb[:], ones512_d)
            nc.sync.dma_start(b2s_sb[:], b2s_d)
            nc.sync.dma_start(masks_sb[:], masks_d.rearrange("o p t -> p o t"))

        def _sums(src_sl, add_engines, sq_engines):
            """s1/s2 column sums with one [1,TB] matmul each: partial sums /
            squares are tree-reduced on the cycled engines first, so the PE
            only contracts a single 128-chunk per statistic. add_engines must
            be DVE/Pool; sq_engines may include nc.scalar (ACT Square)."""
            s1 = ps.tile([1, TB], F32, tag="acc", bufs=2, name="s1")
            s2 = ps.tile([1, TB], F32, tag="acc", bufs=2, name="s2")
            ai, qi = [0], [0]

            def aeng():
                e = add_engines[ai[0] % len(add_engines)]
                ai[0] += 1
                return e

            def sq_into(dst, src):
                e = sq_engines[qi[0] % len(sq_engines)]
                qi[0] += 1
                if e is nc.scalar:
                    nc.scalar.activation(dst, src, AF.Square)
                else:
                    e.tensor_mul(dst, src, src)

            # raw-x tree: (0+1), (2+3), (4+5) -> pairwise combine
            p01 = work.tile([P, TB], F32R, tag="sq", bufs=4, name="p01")
            p23 = work.tile([P, TB], F32R, tag="sq", bufs=4, name="p23")
            p45 = work.tile([P, TB], F32R, tag="sq", bufs=4, name="p45")
            aeng().tensor_add(p01[:], src_sl[:, 0, :], src_sl[:, 1, :])
            aeng().tensor_add(p23[:], src_sl[:, 2, :], src_sl[:, 3, :])
            aeng().tensor_add(p45[:], src_sl[:, 4, :], src_sl[:, 5, :])
            aeng().tensor_add(p01[:], p01[:], p23[:])
            aeng().tensor_add(p01[:], p01[:], p45[:])
            nc.tensor.matmul(s1[:], onesc_sb[:], p01[:], start=True, stop=True)
            # squares tree
            sqs = []
            for jp in range(3):
                a = work.tile([P, TB], F32R, tag="sq", bufs=4, name="sqa")
                b = work.tile([P, TB], F32R, tag="sq", bufs=4, name="sqb")
                sq_into(a[:], src_sl[:, 2 * jp, :])
                sq_into(b[:], src_sl[:, 2 * jp + 1, :])
                aeng().tensor_add(a[:], a[:], b[:])
                sqs.append(a)
            aeng().tensor_add(sqs[0][:], sqs[0][:], sqs[1][:])
            aeng().tensor_add(sqs[0][:], sqs[0][:], sqs[2][:])
            nc.tensor.matmul(s2[:], onesc_sb[:], sqs[0][:],
                             start=True, stop=True)
            return s1, s2

        def _muvar(s1, s2):
            mu = rows.tile([1, TB], F32R, tag="mu", bufs=2)
            t = rows.tile([1, TB], F32R, tag="tmp", bufs=2)
            r = rows.tile([1, TB], F32R, tag="r", bufs=2)
            nc.vector.tensor_scalar_mul(mu[:], s1[:], 1.0 / D)
            nc.vector.tensor_mul(t[:], mu[:], mu[:])
            nc.vector.scalar_tensor_tensor(t[:], s2[:], 1.0 / D, t[:],
                                           ALU.mult, ALU.subtract)
            nc.vector.tensor_scalar_add(t[:], t[:], EPS)
            nc.scalar.activation(t[:], t[:], AF.Sqrt)
            nc.vector.reciprocal(r[:], t[:])
            return mu, r, t

        def ln_stats(src_sl):
            """src_sl: [128, NC, TB] slice. Returns (r, mur) rows in SBUF.
            Avoids ACT entirely (it is exp-saturated during attention)."""
            s1, s2 = _sums(src_sl, (nc.gpsimd, nc.vector),
                           (nc.vector, nc.gpsimd))
            mu, r, _ = _muvar(s1, s2)
            mur = rows.tile([1, TB], F32R, tag="mur", bufs=2)
            nc.vector.tensor_mul(mur[:], mu[:], r[:])
            return r, mur

        def ln1_stats(tb, xt_t):
            """Stats for the folded LN1: returns (mu, bcrs, rcol).

            mu: [1,TB] f32r row (rank-1 correction rhs); bcrs: [P,TB] bf16
            broadcast of 1/sigma (columns); rcol: [P,4] f32 1/sigma along
            partitions for this block's four 128-token s-chunks (V scale).
            """
            if tb == 0:
                adds, sqe = (nc.vector, nc.gpsimd), \
                    (nc.scalar, nc.vector, nc.gpsimd)
            else:
                adds, sqe = (nc.gpsimd, nc.vector), (nc.scalar, nc.gpsimd)
            s1, s2 = _sums(xt_t[:], adds, sqe)
            mu, r, sig = _muvar(s1, s2)
            bcr = ps.tile([P, TB], F32, tag="yt", bufs=2, name="bcs")
            nc.tensor.matmul(bcr[:], onesr_sb[:], r[:], start=True, stop=True)
            bcrs = work.tile([P, TB], BF16, tag="bcrs")
            nc.vector.tensor_copy(bcrs[:], bcr[:])
            rtp = ps.tile([P, TB], F32, tag="yt", bufs=2, name="rtp")
            for si in range(4):
                nc.tensor.matmul(rtp[:, si:si + 1], r[:, si * P:(si + 1) * P],
                                 onesr_sb[:, 0:1], start=True, stop=True)
            rcol = work.tile([P, 4], F32, tag="rcol")
            nc.vector.tensor_copy(rcol[:], rtp[:, 0:4])
            return mu, bcrs, rcol, sig

        def ln_normalize(src_sl, dst_sl, r, mur, g_sb, be_sb, g_row):
            """dst = ((src*g[p])*bc(r) + be[p]) - bc(g[p]*mur)."""
            bcr = ps.tile([P, TB], F32, tag="yt", bufs=2, name="bcs")[:]
            nc.tensor.matmul(bcr, onesr_sb[:], r[:], start=True, stop=True)
            for j in range(NC):
                bc2 = ps.tile([P, TB], F32, tag="acc", bufs=2, name="bc2")
                nc.tensor.matmul(bc2[:], g_row[:, j * P:(j + 1) * P], mur[:],
                                 start=True, stop=True)
                t1 = work.tile([P, TB], F32R, tag="nrm")
                nc.vector.scalar_tensor_tensor(t1[:], src_sl[:, j, :],
                                               g_sb[:, j:j + 1], bcr,
                                               ALU.mult, ALU.mult)
                nc.vector.scalar_tensor_tensor(dst_sl[:, j, :], t1[:],
                                               be_sb[:, j:j + 1], bc2[:],
                                               ALU.add, ALU.subtract)

        # ---------------- Phase 1+2: LN1-folded QKV, software-pipelined -----
        _mark(nc, "ln1")
        es_kqv = ExitStack()
        p_kqv = es_kqv.enter_context(tc.tile_pool(name="p_kqv", bufs=1,
                                                  side="right"))
        kt_sb = p_kqv.tile([P, NC, T], BF16, tag="kt")      # K^T [m, s]
        qt_sb = p_kqv.tile([P, NC, 1024], BF16, tag="qt")   # Q^T [m, t_own]
        v_sb = p_kqv.tile([P, 16, H * 65], BF16, tag="v")   # V_ext [s, (h,65)]
        v_view = v_sb.rearrange("p s (h e) -> p s h e", e=65)
        nc.vector.memset(v_view[:, :, :, 64:65], 1.0)

        es_masks = ExitStack()
        p_masks = es_masks.enter_context(tc.tile_pool(name="p_masks", bufs=1,
                                                      side="right"))
        p_e = es_masks.enter_context(tc.tile_pool(name="p_e", bufs=6,
                                                  side="right"))
        masks_sb = p_masks.tile([P, 4, 2 * TB], BF16, tag="masks")
        # tri chunks (di>0) exp/mask/PV all operate only on the live column
        # sub-range [lo:], so the masked-out columns are never read and need
        # no pre-zeroed tiles.

        es_wqkv = ExitStack()
        p_wqkv = es_wqkv.enter_context(tc.tile_pool(name="p_wqkv", bufs=1,
                                                    side="right"))
        wq_sb = p_wqkv.tile([P, NC, D], BF16, tag="wq")
        wk_sb = p_wqkv.tile([P, NC, D], BF16, tag="wk")
        wv_sb = p_wqkv.tile([P, NC, D], BF16, tag="wv")

        def qkv_for_tb(tb, xt_t, mu, bcrs, rcol, sig):
            """QKV projections straight from raw x^T with LN1 folded in:
            psum accumulates (g1-folded w)@x plus the rank-1 -colsum(w)*mu
            correction; the per-token 1/sigma lands at psum readout (bcrs
            columns for K/Q, rcol per-partition scale for V).
            Accumulators live two-per [P, 2*TB] PSUM tile on the "st" tag
            (idle during phase 1)."""
            tsl = slice(tb * TB, (tb + 1) * TB)
            _mark(nc, "qkv")

            def proj_pair(w_sb, ws_row, wbe_row, dst_sb, mtp):
                acc2 = ps.tile([P, 2 * TB], F32, tag="st", bufs=2, name="ka")
                for half in range(2):
                    mt = 2 * mtp + half
                    msl = slice(mt * P, (mt + 1) * P)
                    seg = acc2[:, half * TB:(half + 1) * TB]
                    for j in range(NC):
                        nc.tensor.matmul(seg, w_sb[:, j, msl], xt_t[:, j, :],
                                         start=(j == 0), stop=False)
                    nc.tensor.matmul(seg, ws_row[:, msl], mu[:],
                                     start=False, stop=(not use_be1))
                    if use_be1:
                        # be-term must survive the *1/sigma readout: rhs=sigma
                        nc.tensor.matmul(seg, wbe_row[:, msl], sig[:],
                                         start=False, stop=True)
                for half in range(2):
                    mt = 2 * mtp + half
                    seg = acc2[:, half * TB:(half + 1) * TB]
                    nc.vector.tensor_mul(dst_sb[:, mt, tsl], seg, bcrs[:])

            for mtp in range(3):
                proj_pair(wk_sb, nwks_sb, wkbe_sb if use_be1 else None,
                          kt_sb, mtp)
            for si in range(4):
                st = tb * 4 + si
                lsl = slice(si * P, (si + 1) * P)
                acc2 = ps.tile([P, 2 * TB], F32, tag="st", bufs=2, name="va")
                for half, fsl, off, w in ((0, slice(0, TB), 0, TB),
                                          (1, slice(TB, D), TB, D - TB)):
                    seg = acc2[:, off:off + w]
                    for j in range(NC):
                        nc.tensor.matmul(seg, xt_t[:, j, lsl],
                                         wv_sb[:, j, fsl],
                                         start=(j == 0), stop=False)
                    nc.tensor.matmul(seg, mu[:, lsl], nwvs_sb[:, fsl],
                                     start=False, stop=(not use_be1))
                    if use_be1:
                        nc.tensor.matmul(seg, sig[:, lsl], wvbe_sb[:, fsl],
                                         start=False, stop=True)
                for half, off, w in ((0, 0, TB), (1, TB, D - TB)):
                    src = acc2[:, off:off + w].rearrange(
                        "p (h e) -> p h e", e=64)
                    h0 = half * 8
                    nc.scalar.activation(
                        v_view[:, st, h0:h0 + w // 64, 0:64], src, AF.Copy,
                        scale=rcol[:, si:si + 1])
            if tb < 2:
                for mtp in range(3):
                    proj_pair(wq_sb, nwqs_sb, wqbe_sb if use_be1 else None,
                              qt_sb, mtp)

        with tc.tile_pool(name="p_xtr", bufs=3) as p_xtr:
            stats = {}
            xts = {}
            for tb in range(NTB):
                tsl = slice(tb * TB, (tb + 1) * TB)
                xt_t = p_xtr.tile([P, NC, TB], F32R, tag="xtr")
                if tb == 0:
                    # tiny consts first (s1's onesc lhsT must not queue
                    # behind the bulk transfers), then the first block split
                    # so the stats tree can start after ~1/3 of it
                    _early_const_dmas()
                    for jj in range(3):
                        nc.sync.dma_start(xt_t[:, 2 * jj:2 * jj + 2, :],
                                          xt_r[:, 2 * jj:2 * jj + 2, tsl])
                else:
                    nc.sync.dma_start(xt_t[:], xt_r[:, :, tsl])
                if tb == 0:
                    nc.sync.dma_start(wk_sb[:],
                                      wkt_d.rearrange("(j p) m -> p j m", p=P))
                    nc.sync.dma_start(wv_sb[:],
                                      wvt_d.rearrange("(j p) m -> p j m", p=P))
                    nc.sync.dma_start(wq_sb[:],
                                      wqt_d.rearrange("(j p) m -> p j m", p=P))
                if tb == 1:
                    _late_const_dmas()
                xts[tb] = xt_t
                # emit qkv(tb-1) before stats(tb): the bulk PE work is ready
                # to run, so the in-order PE stream never parks on the
                # square/sum chain of the next block
                if tb > 0:
                    qkv_for_tb(tb - 1, xts[tb - 1][:], *stats[tb - 1])
                stats[tb] = ln1_stats(tb, xt_t)
            qkv_for_tb(NTB - 1, xts[NTB - 1][:], *stats[NTB - 1])
        es_wqkv.close()

        # ---------------- Phase 3: attention (+ per-slot wo/LN2) -----------
        _mark(nc, "attn")
        p_xp = ctx.enter_context(tc.tile_pool(name="p_xp", bufs=1))
        xp_sb = p_xp.tile([P, NC, 1024], F32R, tag="xp")
        p_xn2 = ctx.enter_context(tc.tile_pool(name="p_xn2", bufs=1))
        xn2_sb = p_xn2.tile([P, NC, 1024], F8, tag="xn2")
        es_yt = ExitStack()
        p_yt = es_yt.enter_context(tc.tile_pool(name="p_yt", bufs=2))
        es_wo = ExitStack()
        p_wo = es_wo.enter_context(tc.tile_pool(name="p_wo", bufs=1))
        wo_sb = p_wo.tile([P, NC, D], BF16, tag="wo")
        nc.sync.dma_start(wo_sb[:], wo_d.rearrange("(j p) m -> p j m", p=P))
        xo_sb = p_wo.tile([P, NC, 1024], F32R, tag="xo")
        nc.sync.dma_start(xo_sb[:], xt_r[:, :, 0:1024])

        if True:
            for sl_i in range(2):
                yt_all = p_yt.tile([P, NC, TB], BF16, tag="yt_all")
                qsl = slice(sl_i * TB, (sl_i + 1) * TB)
                chunks = SLOT_CHUNKS[sl_i]
                for mt in range(NC):
                    yt2 = [ps.tile([65, TB], F32, tag="yt", bufs=2,
                                   name=f"yt_{sl_i}_{mt}_{ph}") for ph in range(2)]
                    for ci, ch in enumerate(chunks):
                        sb_idx = (0 if sl_i == 0 else 8) + ci
                        di = ch - 4 * sl_i
                        tri = 0 <= di < 4
                        lo = 128 * di if tri else 0
                        st2 = ps.tile([P, 2 * TB], F32, tag="st", bufs=2)
                        qsub = slice(sl_i * TB + lo, (sl_i + 1) * TB)
                        for ph in range(2):
                            o = ph * 64
                            nc.tensor.matmul(
                                st2[:, ph * TB + lo:(ph + 1) * TB],
                                kt_sb[o:o + 64, mt, ch * P:(ch + 1) * P],
                                qt_sb[o:o + 64, mt, qsub],
                                start=True, stop=True)
                        e_tile = p_e.tile([P, 2 * TB], BF16, tag="e",
                                          name=f"e_{sl_i}_{mt}_{ci}")
                        e_sb = e_tile[:]
                        if lo:
                            ev = e_sb.rearrange("p (two t) -> p two t",
                                                two=2)[:, :, lo:]
                            sv = st2[:].rearrange("p (two t) -> p two t",
                                                  two=2)[:, :, lo:]
                            mv = masks_sb[:, di, :].rearrange(
                                "p (two t) -> p two t", two=2)[:, :, lo:]
                        else:
                            ev, sv = e_sb, st2[:]
                            mv = masks_sb[:, di, :] if tri else None
                        nc.scalar.activation(
                            ev, sv, AF.Exp,
                            bias=bias_sb[:, sb_idx:sb_idx + 1],
                            scale=scale_sb[:, sb_idx:sb_idx + 1])
                        if tri:
                            nc.vector.tensor_mul(ev, ev, mv)
                        for ph in range(2):
                            h = 2 * mt + ph
                            nc.tensor.matmul(
                                yt2[ph][:, lo:], v_sb[:, ch, h * 65:(h + 1) * 65],
                                e_sb.rearrange("p (two t) -> p two t",
                                               two=2)[:, ph, lo:],
                                start=(ci == 0),
                                stop=(ci == len(chunks) - 1))
                    for ph in range(2):
                        o = ph * 64
                        # copy [65,TB] to SBUF immediately: frees the PSUM
                        # bank so the next mt's PV can start during division
                        yt_sb = work.tile([65, TB], F32R, tag="ydiv")
                        nc.vector.tensor_copy(yt_sb[:], yt2[ph][:])
                        rc = rows.tile([1, TB], F32R, tag="r", bufs=2,
                                       name="rc")
                        nc.vector.reciprocal(rc[:], yt_sb[64:65, :])
                        bc = ps.tile([64, TB], F32, tag="acc", bufs=2,
                                     name="abc")
                        nc.tensor.matmul(bc[:], onesr_sb[:, 0:64],
                                         rc[:], start=True, stop=True)
                        dst = yt_all[o:o + 64, mt, :]
                        nc.vector.tensor_mul(dst, yt_sb[0:64, :], bc[:])
                # w_o projection + residual for this slot (fills exp-waits of
                # the other slot)
                _mark(nc, "wo")
                for ct in range(NC):
                    ao = ps.tile([P, TB], F32, tag="acc", bufs=2, name="ao")
                    for mc in range(NC):
                        nc.tensor.matmul(ao[:],
                                         wo_sb[:, mc, ct * P:(ct + 1) * P],
                                         yt_all[:, mc, :],
                                         start=(mc == 0), stop=(mc == NC - 1))
                    nc.vector.tensor_add(xp_sb[:, ct, qsl],
                                         xo_sb[:, ct, qsl], ao[:])
                _mark(nc, "ln2")
                r2, mur2 = ln_stats(xp_sb[:, :, qsl])
                ln_normalize(xp_sb[:, :, qsl], xn2_sb[:, :, qsl],
                             r2, mur2, g2_sb, be2_sb, g2r_sb)

        es_masks.close()
        es_kqv.close()
        es_wo.close()
        es_yt.close()

        # ---------------- Phase 6: MLP ----------------
        _mark(nc, "mlp")
        w1t_r = w1t_d.rearrange("g (j p) two f -> p g j two f", p=P)
        w2t_r = w2t_d.rearrange("(f p) two c -> p f two c", p=P)
        outt_r = outt_d.rearrange("(j p) t -> p j t", p=P)
        with tc.tile_pool(name="p_h1", bufs=1) as p_h1, \
             tc.tile_pool(name="p_wmlp", bufs=3) as p_wmlp, \
             tc.tile_pool(name="p_w2", bufs=1) as p_w2, \
             tc.tile_pool(name="p_out", bufs=4) as p_out:
            h1_sb = p_h1.tile([P, NF, 1024], F8, tag="h1")
            w2_sb = p_w2.tile([P, NF, 2, D], F8, tag="w2full")
            for ft4 in range(NF // 4):
                w1_t = p_wmlp.tile([P, NC, 2, 4 * P], F8, tag="w1")
                nc.sync.dma_start(w1_t[:], w1t_r[:, ft4])
                # stream all of w2 into SBUF once, interleaved with the w1
                # loads so it is resident before (and reused across) both g
                # passes of the second matmul
                nc.sync.dma_start(w2_sb[:, 4 * ft4:4 * ft4 + 4],
                                  w2t_r[:, 4 * ft4:4 * ft4 + 4, :, :])
                for sub in range(4):
                    ft = 4 * ft4 + sub
                    for tb in range(2):
                        tsl = slice(tb * TB, (tb + 1) * TB)
                        hp = ps.tile([P, TB], F32, tag="acc", bufs=2, name="hp")
                        for j in range(NC):
                            nc.tensor.matmul(
                                hp[:],
                                w1_t[:, j, :, sub * P:(sub + 1) * P],
                                xn2_sb[:, j, tsl].unsqueeze(1)
                                .broadcast_to([P, 2, TB]),
                                start=(j == 0), stop=(j == NC - 1),
                                perf_mode=DR)
                        # h1 = S_H1*relu(psum/S_W1 + b1): scale=S_H1/S_W1,
                        # bias = S_H1*b1 (host-prescaled in b1v)
                        nc.scalar.activation(h1_sb[:, ft, tsl], hp[:], AF.Relu,
                                             bias=b1_sb[:, ft:ft + 1],
                                             scale=S_H1 / S_W1)
            for g in range(2):
                o2s = {}
                for ci, (t, bu) in enumerate((("acc", 2), ("yt", 2))):
                    for tb in range(2):
                        o2s[(ci, tb)] = ps.tile([P, TB], F32, tag=t, bufs=bu,
                                                name=f"o2_{g}_{ci}_{tb}")
                stp = ps.tile([P, 2 * TB], F32, tag="st", bufs=2,
                              name=f"o2st_{g}")
                o2s[(2, 0)] = stp[:, 0:TB]
                o2s[(2, 1)] = stp[:, TB:2 * TB]
                # seed each accumulator with (b2/C_MLP) x ones
                for ci in range(3):
                    ct = g * 3 + ci
                    for tb in range(2):
                        nc.tensor.matmul(o2s[(ci, tb)][:],
                                         b2s_sb[:, ct * P:(ct + 1) * P],
                                         ones512_sb[:],
                                         start=True, stop=False)
                for ft in range(NF):
                    for ci in range(3):
                        ct = g * 3 + ci
                        for tb in range(2):
                            nc.tensor.matmul(
                                o2s[(ci, tb)][:],
                                w2_sb[:, ft, :, ct * P:(ct + 1) * P],
                                h1_sb[:, ft, tb * TB:(tb + 1) * TB]
                                .unsqueeze(1).broadcast_to([P, 2, TB]),
                                start=False, stop=(ft == NF - 1),
                                perf_mode=DR)
                for tb in range(2):
                    tsl = slice(tb * TB, (tb + 1) * TB)
                    for ci in range(3):
                        ct = g * 3 + ci
                        ot = p_out.tile([P, TB], F32, tag="ot",
                                        name=f"ot_{g}_{tb}_{ci}")
                        nc.vector.scalar_tensor_tensor(
                            ot[:], o2s[(ci, tb)][:], C_MLP,
                            xp_sb[:, ct, tsl], ALU.mult, ALU.add)
                        nc.sync.dma_start(outt_r[:, ct, tsl], ot[:])

    nc.compile()
    return nc


def _hilo(w, f8):
    """[..., n] -> [..., 2, n] fp8 (hi, residual-lo) planes."""
    hi = w.astype(f8)
    lo = (w - hi.astype(np.float32)).astype(f8)
    return np.ascontiguousarray(np.stack([hi, lo], axis=-2))


def _host_inputs(X, w_q, w_k, w_v, w_o, W1, b1, W2, b2, g1, be1, g2, be2):
    """Build the 8 per-core input dicts."""
    f32 = np.float32
    import ml_dtypes as _mld
    _f8 = _mld.float8_e4m3
    _bf = _mld.bfloat16
    g1v = np.asarray(g1, f32)
    be1v = np.asarray(be1, f32)
    # LN1 fold: g1 into the QKV weight columns; mean correction rows are the
    # negated column sums; optional be1 rows handle a nonzero LN1 shift
    wqg = np.asarray(w_q, f32).reshape(D, D) * g1v[None, :]
    wkg = np.asarray(w_k, f32).reshape(D, D) * g1v[None, :]
    wvg = np.asarray(w_v, f32).reshape(D, D) * g1v[None, :]
    wqt = np.ascontiguousarray(wqg.T.astype(_bf))
    wkt = np.ascontiguousarray(wkg.T.astype(_bf))
    wvt = np.ascontiguousarray(wvg.T.astype(_bf))
    nwqs = np.ascontiguousarray(-wqg.sum(axis=1).reshape(1, D))
    nwks = np.ascontiguousarray(-wkg.sum(axis=1).reshape(1, D))
    nwvs = np.ascontiguousarray(-wvg.sum(axis=1).reshape(1, D))
    use_be1 = bool(np.any(be1v))
    wo = np.ascontiguousarray(np.asarray(w_o, f32).astype(_bf))
    w1t = None  # bf16, set below
    w2t = None  # bf16, set below
    onesr = np.ones((1, P), f32)
    onesc = np.ones((P, 1), f32)
    onesv = None  # set below after bf16 import
    # 4 canonical self-diagonal masks: mask[k][s, t] = (128k + s <= t)
    import ml_dtypes
    bf16 = ml_dtypes.bfloat16
    masks = np.zeros((4, P, 2 * TB), bf16)
    ar_s = np.arange(P)[:, None]
    ar_t = np.arange(TB)[None, :]
    for k in range(4):
        m = (128 * k + ar_s <= ar_t).astype(bf16)
        masks[k, :, 0:TB] = m
        masks[k, :, TB:2 * TB] = m
    w1t = _hilo(np.asarray(W1, f32).T * S_W1, _f8)   # [D, 2, DFF]
    w1t = np.ascontiguousarray(
        w1t.reshape(D, 2, NF // 4, 4 * P).transpose(2, 0, 1, 3))
    w2t = _hilo(np.asarray(W2, f32).T * S_W2, _f8)

    # per-role exp scale/bias: 24 = 8 (slot0) + 16 (slot1) chunk positions
    sc = {}
    bi = {}
    for role in range(2):
        order = ROLE_ORDER[role]
        s = np.full((24,), 0.125, f32)
        b = np.zeros((24,), f32)
        for sl_i in range(2):
            own_blk = order[sl_i]
            for ci, ch in enumerate(SLOT_CHUNKS[sl_i]):
                idx = (0 if sl_i == 0 else 8) + ci
                pos = ch // 4           # permuted 512-block of this s-chunk
                blk = order[pos]
                if pos == sl_i or blk < own_blk:
                    pass                # diagonal (tri-masked) or past: live
                else:
                    s[idx] = 0.0        # future: dead
                    b[idx] = DEAD
        sc[role] = np.broadcast_to(s, (P, 24)).copy()
        bi[role] = np.broadcast_to(b, (P, 24)).copy()

    g2r = np.asarray(g2, f32).reshape(1, D)
    shared = dict(wqt=wqt, wkt=wkt, wvt=wvt, wo=wo, w1t=w1t, w2t=w2t,
                  g2r=g2r, nwqs=nwqs, nwks=nwks, nwvs=nwvs,
                  onesr=onesr, onesc=onesc, masks=masks,
                  g2v=np.asarray(g2, f32), be2v=np.asarray(be2, f32),
                  b1v=np.asarray(b1, f32) * S_H1,
                  b2s=np.asarray(b2, f32).reshape(1, D) * (S_H1 * S_W2),
                  ones512=np.ones((1, TB), f32))
    if use_be1:
        shared["wqbe"] = (np.asarray(w_q, f32).reshape(D, D) @ be1v
                          ).reshape(1, D).astype(f32)
        shared["wkbe"] = (np.asarray(w_k, f32).reshape(D, D) @ be1v
                          ).reshape(1, D).astype(f32)
        shared["wvbe"] = (np.asarray(w_v, f32).reshape(D, D) @ be1v
                          ).reshape(1, D).astype(f32)

    in_maps = []
    for core in range(8):
        role, b_idx = core // 4, core % 4
        order = ROLE_ORDER[role]
        xb = np.asarray(X[b_idx], f32)          # [T, D]
        xperm = np.concatenate([xb[o * TB:(o + 1) * TB] for o in order], axis=0)
        xt = np.ascontiguousarray(xperm.T)      # [D, T]
        m = dict(shared)
        m["xt"] = xt
        m["scalein"] = sc[role]
        m["biasin"] = bi[role]
        in_maps.append(m)
    return in_maps


def _assemble(results, dtype):
    out = np.empty((B, T, D), dtype)
    for core in range(8):
        role, b_idx = core // 4, core % 4
        order = ROLE_ORDER[role]
        ot = results[core]["outt"]              # [D, 1024]
        for sl_i in range(2):
            blk = order[sl_i]
            out[b_idx, blk * TB:(blk + 1) * TB] = \
                ot[:, sl_i * TB:(sl_i + 1) * TB].T
    return out


def kernel(X, w_q, w_k, w_v, w_o, W1, b1, W2, b2, g1, be1, g2, be2,
           _want_results=False, _trace=False):
    use_be1 = bool(np.any(np.asarray(be1)))
    key = ("nc", use_be1)
    if key not in _cached:
        _cached[key] = _build_nc(use_be1=use_be1)
        _cached["nc"] = _cached[key]
    nc = _cached[key]
    in_maps = _host_inputs(X, w_q, w_k, w_v, w_o, W1, b1, W2, b2,
                           g1, be1, g2, be2)
    res = run_bass_kernel_spmd(nc, in_maps, core_ids=list(range(8)),
                               trace=_trace)
    out = _assemble(res.results, np.asarray(X).dtype)
    if _want_results:
        return out, res
    return out



# revision 35
# speedup vs baseline: 1.0302x; 1.0302x over previous
"""Trainium2 Bass kernel for a dense transformer block (pre-LN attn + MLP).

B=4, T=2048, D=768, H=12 (DH=64), DFF=3072, fp32.

Sharding: 8 cores = 4 batches x 2 roles. Each core processes one batch and
owns 1024 query tokens (two 512-blocks, paired {0,3}/{1,2} for causal load
balance). K/V are computed for the full 2048 tokens on both cores of a batch
(cheap), so there are NO collectives.

SPMD uniformity: all 8 cores run ONE identical NEFF. Causal structure is
carried in DATA, not code:
  - host permutes each batch's token axis to [own0, own1, otherA, otherB]
  - q-slot0 attends s-chunks {0..3, 8..11}; q-slot1 attends s-chunks {0..15}
  - per-(slot,chunk) exp scale/bias inputs select live / dead (zero) chunks
  - 4 canonical triangular masks handle the self-diagonal 512-blocks

Everything on-chip runs in a transposed layout (features on partitions,
tokens on the free axis) so no on-chip transposes are needed; all weight /
input transposes happen on the host in numpy. Matmuls run as float32r
(full PE speed, ~bf16x2 precision). LayerNorm statistics are computed with
ones-column matmuls; per-token stats are broadcast across partitions with
K=1 outer-product matmuls. Softmax denominators come for free from a ones
column appended to V (65-row PV matmul); the divide is folded in after PV.
"""

import sys

sys.path.insert(0, "/opt/trn_rl_repo")

from contextlib import ExitStack

import numpy as np

import concourse.bass as bass
import concourse.mybir as mybir
import concourse.tile as tile
from concourse import bacc
from concourse.bass_utils import run_bass_kernel_spmd

F32 = mybir.dt.float32
F32R = mybir.dt.float32r
AF = mybir.ActivationFunctionType
BF16 = mybir.dt.bfloat16
F8 = mybir.dt.float8e4
DR = mybir.MatmulPerfMode.DoubleRow
ALU = mybir.AluOpType

# fp8 pre-scales (host folds these into the weights; kernel divides out)
S_W1 = 16.0     # W1 stored as 16*W1
S_H1 = 8.0      # h1 stored as 8*relu(.)
S_W2 = 16.0     # W2 stored as 16*W2
C_MLP = 1.0 / (S_H1 * S_W2)          # o2 -> true h@W2.T scale
S_YDIV = 1.0    # PV sums pre-scale before the divide (1 = none)
C_WO = 1.0      # wo-out descale folded into the residual add

H, D, DFF = 12, 768, 3072
DH = 64
B, T = 4, 2048
EPS = 1e-5
P = 128
NC = D // P          # 6 feature chunks
NF = DFF // P        # 24 ff tiles
TB = 512             # token block
NTB = T // TB        # 4 blocks
SLOT_CHUNKS = [[0, 1, 2, 3, 8, 9, 10, 11], list(range(16))]
# role -> permuted block order [own0, own1, restA, restB] (original block ids)
ROLE_ORDER = [[0, 3, 1, 2], [1, 2, 0, 3]]
DEAD = -30000.0      # exp(DEAD) == 0 in fp32

_cached = {}
PHASE_MARKS = []


def _mark(nc, name):
    PHASE_MARKS.append((name, nc.next_id()))


def _build_nc(use_be1=False):
    nc = bacc.Bacc("TRN2", target_bir_lowering=False, debug=False,
                   enable_asserts=False, num_devices=8)

    def din(name, shape, dt=F32R):
        return nc.dram_tensor(name, shape, dt, kind="ExternalInput").ap()

    xt_d = din("xt", [D, T])                 # X[b].T, token-permuted
    wqt_d = din("wqt", [D, D], BF16)         # g1-folded w_q as [c, m]
    wkt_d = din("wkt", [D, D], BF16)
    wvt_d = din("wvt", [D, D], BF16)
    wo_d = din("wo", [D, D], BF16)           # natural [m, c]
    w1t_d = din("w1t", [NF // 4, D, 2, 4 * P], F8)  # [fgrp, c, (hi/lo), f]
    w2t_d = din("w2t", [DFF, 2, D], F8)      # 16*W2.T  [f, (hi/lo), c]
    ones512_d = din("ones512", [1, TB])      # ones row for b2 outer
    b2s_d = din("b2s", [1, D])               # 128*b2 as a row
    onesr_d = din("onesr", [1, P])           # outer-product lhsT
    onesc_d = din("onesc", [P, 1])           # column-sum lhsT
    masks_d = din("masks", [4, P, 2 * TB], BF16)  # tri masks x2 halves
    scalein_d = din("scalein", [P, 24], F32) # exp scale per (slot,chunk)
    biasin_d = din("biasin", [P, 24], F32)   # exp bias per (slot,chunk)
    # LN1 is folded into the QKV path: -colsum(g1-folded w) rows for the
    # rank-1 mean correction (and optional be1-fold rows)
    nwqs_d = din("nwqs", [1, D])
    nwks_d = din("nwks", [1, D])
    nwvs_d = din("nwvs", [1, D])
    if use_be1:
        wqbe_d = din("wqbe", [1, D])
        wkbe_d = din("wkbe", [1, D])
        wvbe_d = din("wvbe", [1, D])
    g2_d = din("g2v", [D], F32)
    be2_d = din("be2v", [D], F32)
    g2r_d = din("g2r", [1, D])
    b1_d = din("b1v", [DFF], F32)

    outt_d = nc.dram_tensor("outt", [D, 1024], F32, kind="ExternalOutput").ap()

    xt_r = xt_d.rearrange("(j p) t -> p j t", p=P)

    with tile.TileContext(nc) as tc, ExitStack() as ctx, \
         nc.allow_low_precision(reason="fp32r/bf16 intermediates are intended"):
        consts = ctx.enter_context(tc.tile_pool(name="consts", bufs=1))
        ps = ctx.enter_context(tc.tile_pool(name="ps", bufs=1, space="PSUM"))
        rows = ctx.enter_context(tc.tile_pool(name="rows", bufs=1))
        work = ctx.enter_context(tc.tile_pool(name="work", bufs=2))

        onesr_sb = consts.tile([1, P], F32R, tag="onesr")
        onesc_sb = consts.tile([P, 1], F32R, tag="onesc")
        scale_sb = consts.tile([P, 24], F32, tag="scalein")
        bias_sb = consts.tile([P, 24], F32, tag="biasin")
        g2_sb = consts.tile([P, NC], F32, tag="g2")
        be2_sb = consts.tile([P, NC], F32, tag="be2")
        g2r_sb = consts.tile([1, D], F32R, tag="g2r")
        b1_sb = consts.tile([P, NF], F32, tag="b1")
        ones512_sb = consts.tile([1, TB], F32R, tag="ones512")
        b2s_sb = consts.tile([1, D], F32R, tag="b2s")
        nwqs_sb = consts.tile([1, D], F32R, tag="nwqs")
        nwks_sb = consts.tile([1, D], F32R, tag="nwks")
        nwvs_sb = consts.tile([1, D], F32R, tag="nwvs")
        if use_be1:
            wqbe_sb = consts.tile([1, D], F32R, tag="wqbe")
            wkbe_sb = consts.tile([1, D], F32R, tag="wkbe")
            wvbe_sb = consts.tile([1, D], F32R, tag="wvbe")

        def _early_const_dmas():
            nc.sync.dma_start(onesc_sb[:], onesc_d)
            nc.sync.dma_start(onesr_sb[:], onesr_d)
            nc.sync.dma_start(nwqs_sb[:], nwqs_d)
            nc.sync.dma_start(nwks_sb[:], nwks_d)
            nc.sync.dma_start(nwvs_sb[:], nwvs_d)
            if use_be1:
                nc.sync.dma_start(wqbe_sb[:], wqbe_d)
                nc.sync.dma_start(wkbe_sb[:], wkbe_d)
                nc.sync.dma_start(wvbe_sb[:], wvbe_d)
                nc.sync.dma_start(ones512_sb[:], ones512_d)

        def _late_const_dmas():
            nc.sync.dma_start(scale_sb[:], scalein_d)
            nc.sync.dma_start(bias_sb[:], biasin_d)
            nc.sync.dma_start(g2r_sb[:], g2r_d)
            for sb, d in ((g2_sb, g2_d), (be2_sb, be2_d)):
                nc.sync.dma_start(sb[:], d.rearrange("(j p) -> p j", p=P))
            nc.sync.dma_start(b1_sb[:], b1_d.rearrange("(j p) -> p j", p=P))
            if not use_be1:
                nc.sync.dma_start(ones512_sb[:], ones512_d)
            nc.sync.dma_start(b2s_sb[:], b2s_d)
            nc.sync.dma_start(masks_sb[:], masks_d.rearrange("o p t -> p o t"))

        def _sums(src_sl, sq_engines):
            """s1/s2 column-sum matmuls; squares cycled over sq_engines."""
            s1 = ps.tile([1, TB], F32, tag="acc", bufs=2, name="s1")
            s2 = ps.tile([1, TB], F32, tag="acc", bufs=2, name="s2")
            for j in range(NC):
                nc.tensor.matmul(s1[:], onesc_sb[:], src_sl[:, j, :],
                                 start=(j == 0), stop=(j == NC - 1))
            for j in range(NC):
                sq = work.tile([P, TB], F32R, tag="sq", bufs=3)
                eng = sq_engines[j % len(sq_engines)]
                if eng is nc.scalar:
                    nc.scalar.activation(sq[:], src_sl[:, j, :], AF.Square)
                else:
                    eng.tensor_mul(sq[:], src_sl[:, j, :], src_sl[:, j, :])
                nc.tensor.matmul(s2[:], onesc_sb[:], sq[:],
                                 start=(j == 0), stop=(j == NC - 1))
            return s1, s2

        def _muvar(s1, s2):
            mu = rows.tile([1, TB], F32R, tag="mu", bufs=2)
            t = rows.tile([1, TB], F32R, tag="tmp", bufs=2)
            r = rows.tile([1, TB], F32R, tag="r", bufs=2)
            nc.vector.tensor_scalar_mul(mu[:], s1[:], 1.0 / D)
            nc.vector.tensor_mul(t[:], mu[:], mu[:])
            nc.vector.scalar_tensor_tensor(t[:], s2[:], 1.0 / D, t[:],
                                           ALU.mult, ALU.subtract)
            nc.vector.tensor_scalar_add(t[:], t[:], EPS)
            nc.scalar.activation(t[:], t[:], AF.Sqrt)
            nc.vector.reciprocal(r[:], t[:])
            return mu, r, t

        def ln_stats(src_sl):
            """src_sl: [128, NC, TB] slice. Returns (r, mur) rows in SBUF.
            Avoids ACT entirely (it is exp-saturated during attention)."""
            s1, s2 = _sums(src_sl, (nc.vector, nc.gpsimd))
            mu, r, _ = _muvar(s1, s2)
            mur = rows.tile([1, TB], F32R, tag="mur", bufs=2)
            nc.vector.tensor_mul(mur[:], mu[:], r[:])
            return r, mur

        def ln1_stats(tb, xt_t):
            """Stats for the folded LN1: returns (mu, bcrs, rcol).

            mu: [1,TB] f32r row (rank-1 correction rhs); bcrs: [P,TB] bf16
            broadcast of 1/sigma (columns); rcol: [P,4] f32 1/sigma along
            partitions for this block's four 128-token s-chunks (V scale).
            """
            sqe = (nc.scalar, nc.vector, nc.gpsimd) if tb == 0 \
                else (nc.scalar, nc.gpsimd)
            s1, s2 = _sums(xt_t[:], sqe)
            mu, r, sig = _muvar(s1, s2)
            bcr = ps.tile([P, TB], F32, tag="yt", bufs=2, name="bcs")
            nc.tensor.matmul(bcr[:], onesr_sb[:], r[:], start=True, stop=True)
            bcrs = work.tile([P, TB], BF16, tag="bcrs")
            nc.vector.tensor_copy(bcrs[:], bcr[:])
            rtp = ps.tile([P, TB], F32, tag="yt", bufs=2, name="rtp")
            for si in range(4):
                nc.tensor.matmul(rtp[:, si:si + 1], r[:, si * P:(si + 1) * P],
                                 onesr_sb[:, 0:1], start=True, stop=True)
            rcol = work.tile([P, 4], F32, tag="rcol")
            nc.vector.tensor_copy(rcol[:], rtp[:, 0:4])
            return mu, bcrs, rcol, sig

        def ln_normalize(src_sl, dst_sl, r, mur, g_sb, be_sb, g_row):
            """dst = ((src*g[p])*bc(r) + be[p]) - bc(g[p]*mur)."""
            bcr = ps.tile([P, TB], F32, tag="yt", bufs=2, name="bcs")[:]
            nc.tensor.matmul(bcr, onesr_sb[:], r[:], start=True, stop=True)
            for j in range(NC):
                bc2 = ps.tile([P, TB], F32, tag="acc", bufs=2, name="bc2")
                nc.tensor.matmul(bc2[:], g_row[:, j * P:(j + 1) * P], mur[:],
                                 start=True, stop=True)
                t1 = work.tile([P, TB], F32R, tag="nrm")
                nc.vector.scalar_tensor_tensor(t1[:], src_sl[:, j, :],
                                               g_sb[:, j:j + 1], bcr,
                                               ALU.mult, ALU.mult)
                nc.vector.scalar_tensor_tensor(dst_sl[:, j, :], t1[:],
                                               be_sb[:, j:j + 1], bc2[:],
                                               ALU.add, ALU.subtract)

        # ---------------- Phase 1+2: LN1-folded QKV, software-pipelined -----
        _mark(nc, "ln1")
        es_kqv = ExitStack()
        p_kqv = es_kqv.enter_context(tc.tile_pool(name="p_kqv", bufs=1,
                                                  side="right"))
        kt_sb = p_kqv.tile([P, NC, T], BF16, tag="kt")      # K^T [m, s]
        qt_sb = p_kqv.tile([P, NC, 1024], BF16, tag="qt")   # Q^T [m, t_own]
        v_sb = p_kqv.tile([P, 16, H * 65], BF16, tag="v")   # V_ext [s, (h,65)]
        v_view = v_sb.rearrange("p s (h e) -> p s h e", e=65)
        nc.vector.memset(v_view[:, :, :, 64:65], 1.0)

        es_masks = ExitStack()
        p_masks = es_masks.enter_context(tc.tile_pool(name="p_masks", bufs=1,
                                                      side="right"))
        p_e = es_masks.enter_context(tc.tile_pool(name="p_e", bufs=6,
                                                  side="right"))
        masks_sb = p_masks.tile([P, 4, 2 * TB], BF16, tag="masks")
        # tri chunks (di>0) exp/mask/PV all operate only on the live column
        # sub-range [lo:], so the masked-out columns are never read and need
        # no pre-zeroed tiles.

        es_wqkv = ExitStack()
        p_wqkv = es_wqkv.enter_context(tc.tile_pool(name="p_wqkv", bufs=1,
                                                    side="right"))
        wq_sb = p_wqkv.tile([P, NC, D], BF16, tag="wq")
        wk_sb = p_wqkv.tile([P, NC, D], BF16, tag="wk")
        wv_sb = p_wqkv.tile([P, NC, D], BF16, tag="wv")

        def qkv_for_tb(tb, xt_t, mu, bcrs, rcol, sig):
            """QKV projections straight from raw x^T with LN1 folded in:
            psum accumulates (g1-folded w)@x plus the rank-1 -colsum(w)*mu
            correction; the per-token 1/sigma lands at psum readout (bcrs
            columns for K/Q, rcol per-partition scale for V).
            Accumulators live two-per [P, 2*TB] PSUM tile on the "st" tag
            (idle during phase 1)."""
            tsl = slice(tb * TB, (tb + 1) * TB)
            _mark(nc, "qkv")

            def proj_pair(w_sb, ws_row, wbe_row, dst_sb, mtp):
                acc2 = ps.tile([P, 2 * TB], F32, tag="st", bufs=2, name="ka")
                for half in range(2):
                    mt = 2 * mtp + half
                    msl = slice(mt * P, (mt + 1) * P)
                    seg = acc2[:, half * TB:(half + 1) * TB]
                    for j in range(NC):
                        nc.tensor.matmul(seg, w_sb[:, j, msl], xt_t[:, j, :],
                                         start=(j == 0), stop=False)
                    nc.tensor.matmul(seg, ws_row[:, msl], mu[:],
                                     start=False, stop=(not use_be1))
                    if use_be1:
                        # be-term must survive the *1/sigma readout: rhs=sigma
                        nc.tensor.matmul(seg, wbe_row[:, msl], sig[:],
                                         start=False, stop=True)
                for half in range(2):
                    mt = 2 * mtp + half
                    seg = acc2[:, half * TB:(half + 1) * TB]
                    nc.vector.tensor_mul(dst_sb[:, mt, tsl], seg, bcrs[:])

            for mtp in range(3):
                proj_pair(wk_sb, nwks_sb, wkbe_sb if use_be1 else None,
                          kt_sb, mtp)
            for si in range(4):
                st = tb * 4 + si
                lsl = slice(si * P, (si + 1) * P)
                acc2 = ps.tile([P, 2 * TB], F32, tag="st", bufs=2, name="va")
                for half, fsl, off, w in ((0, slice(0, TB), 0, TB),
                                          (1, slice(TB, D), TB, D - TB)):
                    seg = acc2[:, off:off + w]
                    for j in range(NC):
                        nc.tensor.matmul(seg, xt_t[:, j, lsl],
                                         wv_sb[:, j, fsl],
                                         start=(j == 0), stop=False)
                    nc.tensor.matmul(seg, mu[:, lsl], nwvs_sb[:, fsl],
                                     start=False, stop=(not use_be1))
                    if use_be1:
                        nc.tensor.matmul(seg, sig[:, lsl], wvbe_sb[:, fsl],
                                         start=False, stop=True)
                for half, off, w in ((0, 0, TB), (1, TB, D - TB)):
                    src = acc2[:, off:off + w].rearrange(
                        "p (h e) -> p h e", e=64)
                    h0 = half * 8
                    nc.scalar.activation(
                        v_view[:, st, h0:h0 + w // 64, 0:64], src, AF.Copy,
                        scale=rcol[:, si:si + 1])
            if tb < 2:
                for mtp in range(3):
                    proj_pair(wq_sb, nwqs_sb, wqbe_sb if use_be1 else None,
                              qt_sb, mtp)

        with tc.tile_pool(name="p_xtr", bufs=3) as p_xtr:
            stats = {}
            xts = {}
            for tb in range(NTB):
                tsl = slice(tb * TB, (tb + 1) * TB)
                xt_t = p_xtr.tile([P, NC, TB], F32R, tag="xtr")
                if tb == 0:
                    # tiny consts first (s1's onesc lhsT must not queue
                    # behind the bulk transfers), then the first block split
                    # so the stats tree can start after ~1/3 of it
                    _early_const_dmas()
                    for jj in range(3):
                        nc.sync.dma_start(xt_t[:, 2 * jj:2 * jj + 2, :],
                                          xt_r[:, 2 * jj:2 * jj + 2, tsl])
                else:
                    nc.sync.dma_start(xt_t[:], xt_r[:, :, tsl])
                if tb == 0:
                    nc.sync.dma_start(wk_sb[:],
                                      wkt_d.rearrange("(j p) m -> p j m", p=P))
                    nc.sync.dma_start(wv_sb[:],
                                      wvt_d.rearrange("(j p) m -> p j m", p=P))
                    nc.sync.dma_start(wq_sb[:],
                                      wqt_d.rearrange("(j p) m -> p j m", p=P))
                if tb == 1:
                    _late_const_dmas()
                xts[tb] = xt_t
                # emit qkv(tb-1) before stats(tb): the bulk PE work is ready
                # to run, so the in-order PE stream never parks on the
                # square/sum chain of the next block
                if tb > 0:
                    qkv_for_tb(tb - 1, xts[tb - 1][:], *stats[tb - 1])
                stats[tb] = ln1_stats(tb, xt_t)
            qkv_for_tb(NTB - 1, xts[NTB - 1][:], *stats[NTB - 1])
        es_wqkv.close()

        # ---------------- Phase 3: attention (+ per-slot wo/LN2) -----------
        _mark(nc, "attn")
        p_xp = ctx.enter_context(tc.tile_pool(name="p_xp", bufs=1))
        xp_sb = p_xp.tile([P, NC, 1024], F32R, tag="xp")
        p_xn2 = ctx.enter_context(tc.tile_pool(name="p_xn2", bufs=1))
        xn2_sb = p_xn2.tile([P, NC, 1024], F8, tag="xn2")
        es_yt = ExitStack()
        p_yt = es_yt.enter_context(tc.tile_pool(name="p_yt", bufs=2))
        es_wo = ExitStack()
        p_wo = es_wo.enter_context(tc.tile_pool(name="p_wo", bufs=1))
        wo_sb = p_wo.tile([P, NC, D], BF16, tag="wo")
        nc.sync.dma_start(wo_sb[:], wo_d.rearrange("(j p) m -> p j m", p=P))
        xo_sb = p_wo.tile([P, NC, 1024], F32R, tag="xo")
        nc.sync.dma_start(xo_sb[:], xt_r[:, :, 0:1024])

        if True:
            for sl_i in range(2):
                yt_all = p_yt.tile([P, NC, TB], BF16, tag="yt_all")
                qsl = slice(sl_i * TB, (sl_i + 1) * TB)
                chunks = SLOT_CHUNKS[sl_i]
                for mt in range(NC):
                    yt2 = [ps.tile([65, TB], F32, tag="yt", bufs=2,
                                   name=f"yt_{sl_i}_{mt}_{ph}") for ph in range(2)]
                    for ci, ch in enumerate(chunks):
                        sb_idx = (0 if sl_i == 0 else 8) + ci
                        di = ch - 4 * sl_i
                        tri = 0 <= di < 4
                        lo = 128 * di if tri else 0
                        st2 = ps.tile([P, 2 * TB], F32, tag="st", bufs=2)
                        qsub = slice(sl_i * TB + lo, (sl_i + 1) * TB)
                        for ph in range(2):
                            o = ph * 64
                            nc.tensor.matmul(
                                st2[:, ph * TB + lo:(ph + 1) * TB],
                                kt_sb[o:o + 64, mt, ch * P:(ch + 1) * P],
                                qt_sb[o:o + 64, mt, qsub],
                                start=True, stop=True)
                        e_tile = p_e.tile([P, 2 * TB], BF16, tag="e",
                                          name=f"e_{sl_i}_{mt}_{ci}")
                        e_sb = e_tile[:]
                        if lo:
                            ev = e_sb.rearrange("p (two t) -> p two t",
                                                two=2)[:, :, lo:]
                            sv = st2[:].rearrange("p (two t) -> p two t",
                                                  two=2)[:, :, lo:]
                            mv = masks_sb[:, di, :].rearrange(
                                "p (two t) -> p two t", two=2)[:, :, lo:]
                        else:
                            ev, sv = e_sb, st2[:]
                            mv = masks_sb[:, di, :] if tri else None
                        nc.scalar.activation(
                            ev, sv, AF.Exp,
                            bias=bias_sb[:, sb_idx:sb_idx + 1],
                            scale=scale_sb[:, sb_idx:sb_idx + 1])
                        if tri:
                            nc.vector.tensor_mul(ev, ev, mv)
                        for ph in range(2):
                            h = 2 * mt + ph
                            nc.tensor.matmul(
                                yt2[ph][:, lo:], v_sb[:, ch, h * 65:(h + 1) * 65],
                                e_sb.rearrange("p (two t) -> p two t",
                                               two=2)[:, ph, lo:],
                                start=(ci == 0),
                                stop=(ci == len(chunks) - 1))
                    for ph in range(2):
                        o = ph * 64
                        # copy [65,TB] to SBUF immediately: frees the PSUM
                        # bank so the next mt's PV can start during division
                        yt_sb = work.tile([65, TB], F32R, tag="ydiv")
                        nc.vector.tensor_copy(yt_sb[:], yt2[ph][:])
                        rc = rows.tile([1, TB], F32R, tag="r", bufs=2,
                                       name="rc")
                        nc.vector.reciprocal(rc[:], yt_sb[64:65, :])
                        bc = ps.tile([64, TB], F32, tag="acc", bufs=2,
                                     name="abc")
                        nc.tensor.matmul(bc[:], onesr_sb[:, 0:64],
                                         rc[:], start=True, stop=True)
                        dst = yt_all[o:o + 64, mt, :]
                        nc.vector.tensor_mul(dst, yt_sb[0:64, :], bc[:])
                # w_o projection + residual for this slot (fills exp-waits of
                # the other slot)
                _mark(nc, "wo")
                for ct in range(NC):
                    ao = ps.tile([P, TB], F32, tag="acc", bufs=2, name="ao")
                    for mc in range(NC):
                        nc.tensor.matmul(ao[:],
                                         wo_sb[:, mc, ct * P:(ct + 1) * P],
                                         yt_all[:, mc, :],
                                         start=(mc == 0), stop=(mc == NC - 1))
                    nc.vector.tensor_add(xp_sb[:, ct, qsl],
                                         xo_sb[:, ct, qsl], ao[:])
                _mark(nc, "ln2")
                r2, mur2 = ln_stats(xp_sb[:, :, qsl])
                ln_normalize(xp_sb[:, :, qsl], xn2_sb[:, :, qsl],
                             r2, mur2, g2_sb, be2_sb, g2r_sb)

        es_masks.close()
        es_kqv.close()
        es_wo.close()
        es_yt.close()

        # ---------------- Phase 6: MLP ----------------
        _mark(nc, "mlp")
        w1t_r = w1t_d.rearrange("g (j p) two f -> p g j two f", p=P)
        w2t_r = w2t_d.rearrange("(f p) two c -> p f two c", p=P)
        outt_r = outt_d.rearrange("(j p) t -> p j t", p=P)
        with tc.tile_pool(name="p_h1", bufs=1) as p_h1, \
             tc.tile_pool(name="p_wmlp", bufs=3) as p_wmlp, \
             tc.tile_pool(name="p_w2", bufs=1) as p_w2, \
             tc.tile_pool(name="p_out", bufs=4) as p_out:
            h1_sb = p_h1.tile([P, NF, 1024], F8, tag="h1")
            w2_sb = p_w2.tile([P, NF, 2, D], F8, tag="w2full")
            for ft4 in range(NF // 4):
                w1_t = p_wmlp.tile([P, NC, 2, 4 * P], F8, tag="w1")
                nc.sync.dma_start(w1_t[:], w1t_r[:, ft4])
                # stream all of w2 into SBUF once, interleaved with the w1
                # loads so it is resident before (and reused across) both g
                # passes of the second matmul
                nc.sync.dma_start(w2_sb[:, 4 * ft4:4 * ft4 + 4],
                                  w2t_r[:, 4 * ft4:4 * ft4 + 4, :, :])
                for sub in range(4):
                    ft = 4 * ft4 + sub
                    for tb in range(2):
                        tsl = slice(tb * TB, (tb + 1) * TB)
                        hp = ps.tile([P, TB], F32, tag="acc", bufs=2, name="hp")
                        for j in range(NC):
                            nc.tensor.matmul(
                                hp[:],
                                w1_t[:, j, :, sub * P:(sub + 1) * P],
                                xn2_sb[:, j, tsl].unsqueeze(1)
                                .broadcast_to([P, 2, TB]),
                                start=(j == 0), stop=(j == NC - 1),
                                perf_mode=DR)
                        # h1 = S_H1*relu(psum/S_W1 + b1): scale=S_H1/S_W1,
                        # bias = S_H1*b1 (host-prescaled in b1v)
                        nc.scalar.activation(h1_sb[:, ft, tsl], hp[:], AF.Relu,
                                             bias=b1_sb[:, ft:ft + 1],
                                             scale=S_H1 / S_W1)
            for g in range(2):
                o2s = {}
                for ci, (t, bu) in enumerate((("acc", 2), ("yt", 2))):
                    for tb in range(2):
                        o2s[(ci, tb)] = ps.tile([P, TB], F32, tag=t, bufs=bu,
                                                name=f"o2_{g}_{ci}_{tb}")
                stp = ps.tile([P, 2 * TB], F32, tag="st", bufs=2,
                              name=f"o2st_{g}")
                o2s[(2, 0)] = stp[:, 0:TB]
                o2s[(2, 1)] = stp[:, TB:2 * TB]
                # seed each accumulator with (b2/C_MLP) x ones
                for ci in range(3):
                    ct = g * 3 + ci
                    for tb in range(2):
                        nc.tensor.matmul(o2s[(ci, tb)][:],
                                         b2s_sb[:, ct * P:(ct + 1) * P],
                                         ones512_sb[:],
                                         start=True, stop=False)
                for ft in range(NF):
                    for ci in range(3):
                        ct = g * 3 + ci
                        for tb in range(2):
                            nc.tensor.matmul(
                                o2s[(ci, tb)][:],
                                w2_sb[:, ft, :, ct * P:(ct + 1) * P],
                                h1_sb[:, ft, tb * TB:(tb + 1) * TB]
                                .unsqueeze(1).broadcast_to([P, 2, TB]),
                                start=False, stop=(ft == NF - 1),
                                perf_mode=DR)
                for tb in range(2):
                    tsl = slice(tb * TB, (tb + 1) * TB)
                    for ci in range(3):
                        ct = g * 3 + ci
                        ot = p_out.tile([P, TB], F32, tag="ot",
                                        name=f"ot_{g}_{tb}_{ci}")
                        nc.vector.scalar_tensor_tensor(
                            ot[:], o2s[(ci, tb)][:], C_MLP,
                            xp_sb[:, ct, tsl], ALU.mult, ALU.add)
                        nc.sync.dma_start(outt_r[:, ct, tsl], ot[:])

    nc.compile()
    return nc


def _hilo(w, f8):
    """[..., n] -> [..., 2, n] fp8 (hi, residual-lo) planes."""
    hi = w.astype(f8)
    lo = (w - hi.astype(np.float32)).astype(f8)
    return np.ascontiguousarray(np.stack([hi, lo], axis=-2))


def _host_inputs(X, w_q, w_k, w_v, w_o, W1, b1, W2, b2, g1, be1, g2, be2):
    """Build the 8 per-core input dicts."""
    f32 = np.float32
    import ml_dtypes as _mld
    _f8 = _mld.float8_e4m3
    _bf = _mld.bfloat16
    g1v = np.asarray(g1, f32)
    be1v = np.asarray(be1, f32)
    # LN1 fold: g1 into the QKV weight columns; mean correction rows are the
    # negated column sums; optional be1 rows handle a nonzero LN1 shift
    wqg = np.asarray(w_q, f32).reshape(D, D) * g1v[None, :]
    wkg = np.asarray(w_k, f32).reshape(D, D) * g1v[None, :]
    wvg = np.asarray(w_v, f32).reshape(D, D) * g1v[None, :]
    wqt = np.ascontiguousarray(wqg.T.astype(_bf))
    wkt = np.ascontiguousarray(wkg.T.astype(_bf))
    wvt = np.ascontiguousarray(wvg.T.astype(_bf))
    nwqs = np.ascontiguousarray(-wqg.sum(axis=1).reshape(1, D))
    nwks = np.ascontiguousarray(-wkg.sum(axis=1).reshape(1, D))
    nwvs = np.ascontiguousarray(-wvg.sum(axis=1).reshape(1, D))
    use_be1 = bool(np.any(be1v))
    wo = np.ascontiguousarray(np.asarray(w_o, f32).astype(_bf))
    w1t = None  # bf16, set below
    w2t = None  # bf16, set below
    onesr = np.ones((1, P), f32)
    onesc = np.ones((P, 1), f32)
    onesv = None  # set below after bf16 import
    # 4 canonical self-diagonal masks: mask[k][s, t] = (128k + s <= t)
    import ml_dtypes
    bf16 = ml_dtypes.bfloat16
    masks = np.zeros((4, P, 2 * TB), bf16)
    ar_s = np.arange(P)[:, None]
    ar_t = np.arange(TB)[None, :]
    for k in range(4):
        m = (128 * k + ar_s <= ar_t).astype(bf16)
        masks[k, :, 0:TB] = m
        masks[k, :, TB:2 * TB] = m
    w1t = _hilo(np.asarray(W1, f32).T * S_W1, _f8)   # [D, 2, DFF]
    w1t = np.ascontiguousarray(
        w1t.reshape(D, 2, NF // 4, 4 * P).transpose(2, 0, 1, 3))
    w2t = _hilo(np.asarray(W2, f32).T * S_W2, _f8)

    # per-role exp scale/bias: 24 = 8 (slot0) + 16 (slot1) chunk positions
    sc = {}
    bi = {}
    for role in range(2):
        order = ROLE_ORDER[role]
        s = np.full((24,), 0.125, f32)
        b = np.zeros((24,), f32)
        for sl_i in range(2):
            own_blk = order[sl_i]
            for ci, ch in enumerate(SLOT_CHUNKS[sl_i]):
                idx = (0 if sl_i == 0 else 8) + ci
                pos = ch // 4           # permuted 512-block of this s-chunk
                blk = order[pos]
                if pos == sl_i or blk < own_blk:
                    pass                # diagonal (tri-masked) or past: live
                else:
                    s[idx] = 0.0        # future: dead
                    b[idx] = DEAD
        sc[role] = np.broadcast_to(s, (P, 24)).copy()
        bi[role] = np.broadcast_to(b, (P, 24)).copy()

    g2r = np.asarray(g2, f32).reshape(1, D)
    shared = dict(wqt=wqt, wkt=wkt, wvt=wvt, wo=wo, w1t=w1t, w2t=w2t,
                  g2r=g2r, nwqs=nwqs, nwks=nwks, nwvs=nwvs,
                  onesr=onesr, onesc=onesc, masks=masks,
                  g2v=np.asarray(g2, f32), be2v=np.asarray(be2, f32),
                  b1v=np.asarray(b1, f32) * S_H1,
                  b2s=np.asarray(b2, f32).reshape(1, D) * (S_H1 * S_W2),
                  ones512=np.ones((1, TB), f32))
    if use_be1:
        shared["wqbe"] = (np.asarray(w_q, f32).reshape(D, D) @ be1v
                          ).reshape(1, D).astype(f32)
        shared["wkbe"] = (np.asarray(w_k, f32).reshape(D, D) @ be1v
                          ).reshape(1, D).astype(f32)
        shared["wvbe"] = (np.asarray(w_v, f32).reshape(D, D) @ be1v
                          ).reshape(1, D).astype(f32)

    in_maps = []
    for core in range(8):
        role, b_idx = core // 4, core % 4
        order = ROLE_ORDER[role]
        xb = np.asarray(X[b_idx], f32)          # [T, D]
        xperm = np.concatenate([xb[o * TB:(o + 1) * TB] for o in order], axis=0)
        xt = np.ascontiguousarray(xperm.T)      # [D, T]
        m = dict(shared)
        m["xt"] = xt
        m["scalein"] = sc[role]
        m["biasin"] = bi[role]
        in_maps.append(m)
    return in_maps


def _assemble(results, dtype):
    out = np.empty((B, T, D), dtype)
    for core in range(8):
        role, b_idx = core // 4, core % 4
        order = ROLE_ORDER[role]
        ot = results[core]["outt"]              # [D, 1024]
        for sl_i in range(2):
            blk = order[sl_i]
            out[b_idx, blk * TB:(blk + 1) * TB] = \
                ot[:, sl_i * TB:(sl_i + 1) * TB].T
    return out


def kernel(X, w_q, w_k, w_v, w_o, W1, b1, W2, b2, g1, be1, g2, be2,
           _want_results=False, _trace=False):
    use_be1 = bool(np.any(np.asarray(be1)))
    key = ("nc", use_be1)
    if key not in _cached:
        _cached[key] = _build_nc(use_be1=use_be1)
        _cached["nc"] = _cached[key]
    nc = _cached[key]
    in_maps = _host_inputs(X, w_q, w_k, w_v, w_o, W1, b1, W2, b2,
                           g1, be1, g2, be2)
    res = run_bass_kernel_spmd(nc, in_maps, core_ids=list(range(8)),
                               trace=_trace)
    out = _assemble(res.results, np.asarray(X).dtype)
    if _want_results:
        return out, res
    return out



# revision 47
# speedup vs baseline: 1.0364x; 1.0061x over previous
"""Trainium2 Bass kernel for a dense transformer block (pre-LN attn + MLP).

B=4, T=2048, D=768, H=12 (DH=64), DFF=3072, fp32.

Sharding: 8 cores = 4 batches x 2 roles. Each core processes one batch and
owns 1024 query tokens (two 512-blocks, paired {0,3}/{1,2} for causal load
balance). K/V are computed for the full 2048 tokens on both cores of a batch
(cheap), so there are NO collectives.

SPMD uniformity: all 8 cores run ONE identical NEFF. Causal structure is
carried in DATA, not code:
  - host permutes each batch's token axis to [own0, own1, otherA, otherB]
  - q-slot0 attends s-chunks {0..3, 8..11}; q-slot1 attends s-chunks {0..15}
  - per-(slot,chunk) exp scale/bias inputs select live / dead (zero) chunks
  - 4 canonical triangular masks handle the self-diagonal 512-blocks

Everything on-chip runs in a transposed layout (features on partitions,
tokens on the free axis) so no on-chip transposes are needed; all weight /
input transposes happen on the host in numpy. Matmuls run as float32r
(full PE speed, ~bf16x2 precision). LayerNorm statistics are computed with
ones-column matmuls; per-token stats are broadcast across partitions with
K=1 outer-product matmuls. Softmax denominators come for free from a ones
column appended to V (65-row PV matmul); the divide is folded in after PV.
"""

import sys

sys.path.insert(0, "/opt/trn_rl_repo")

from contextlib import ExitStack

import numpy as np

import concourse.bass as bass
import concourse.mybir as mybir
import concourse.tile as tile
from concourse import bacc
from concourse.bass_utils import run_bass_kernel_spmd

F32 = mybir.dt.float32
F32R = mybir.dt.float32r
AF = mybir.ActivationFunctionType
BF16 = mybir.dt.bfloat16
F8 = mybir.dt.float8e4
DR = mybir.MatmulPerfMode.DoubleRow
ALU = mybir.AluOpType

# fp8 pre-scales (host folds these into the weights; kernel divides out)
S_W1 = 16.0     # W1 stored as 16*W1
S_H1 = 8.0      # h1 stored as 8*relu(.)
S_W2 = 16.0     # W2 stored as 16*W2
C_MLP = 1.0 / (S_H1 * S_W2)          # o2 -> true h@W2.T scale
S_YDIV = 1.0    # PV sums pre-scale before the divide (1 = none)
C_WO = 1.0      # wo-out descale folded into the residual add

H, D, DFF = 12, 768, 3072
DH = 64
B, T = 4, 2048
EPS = 1e-5
P = 128
NC = D // P          # 6 feature chunks
NF = DFF // P        # 24 ff tiles
TB = 512             # token block
NTB = T // TB        # 4 blocks
SLOT_CHUNKS = [[0, 1, 2, 3, 8, 9, 10, 11], list(range(16))]
# role -> permuted block order [own0, own1, restA, restB] (original block ids)
ROLE_ORDER = [[0, 3, 1, 2], [1, 2, 0, 3]]
DEAD = -30000.0      # exp(DEAD) == 0 in fp32

_cached = {}
PHASE_MARKS = []


def _mark(nc, name):
    PHASE_MARKS.append((name, nc.next_id()))


def _build_nc(use_be1=False):
    nc = bacc.Bacc("TRN2", target_bir_lowering=False, debug=False,
                   enable_asserts=False, num_devices=8)

    def din(name, shape, dt=F32R):
        return nc.dram_tensor(name, shape, dt, kind="ExternalInput").ap()

    xt_d = din("xt", [D, T])                 # X[b].T, token-permuted (f32)
    xtb_d = din("xtb", [D, T], BF16)         # same, bf16 (QKV matmul input)
    wqt_d = din("wqt", [D, D], BF16)         # g1-folded w_q as [c, m]
    wkt_d = din("wkt", [D, D], BF16)
    wvt_d = din("wvt", [D, D], BF16)
    wo_d = din("wo", [D, D], BF16)           # natural [m, c]
    w1t_d = din("w1t", [NF // 4, D, 2, 4 * P], F8)  # [fgrp, c, (hi/lo), f]
    w2t_d = din("w2t", [DFF, 2, D], F8)      # 16*W2.T  [f, (hi/lo), c]
    ones512_d = din("ones512", [1, TB])      # ones row for b2 outer
    b2s_d = din("b2s", [1, D])               # 128*b2 as a row
    onesr_d = din("onesr", [1, P])           # outer-product lhsT
    onesc_d = din("onesc", [P, 1])           # column-sum lhsT (f32)
    onescb_d = din("onescb", [P, 1], BF16)   # column-sum lhsT (bf16)
    masks_d = din("masks", [4, P, 2 * TB], BF16)  # tri masks x2 halves
    scalein_d = din("scalein", [P, 24], F32) # exp scale per (slot,chunk)
    biasin_d = din("biasin", [P, 24], F32)   # exp bias per (slot,chunk)
    # LN1 is folded into the QKV path: -colsum(g1-folded w) rows for the
    # rank-1 mean correction (and optional be1-fold rows)
    nwqs_d = din("nwqs", [1, D])
    nwks_d = din("nwks", [1, D])
    nwvs_d = din("nwvs", [1, D])
    if use_be1:
        wqbe_d = din("wqbe", [1, D])
        wkbe_d = din("wkbe", [1, D])
        wvbe_d = din("wvbe", [1, D])
    g2_d = din("g2v", [D], F32)
    be2_d = din("be2v", [D], F32)
    g2r_d = din("g2r", [1, D])
    b1_d = din("b1v", [DFF], F32)

    outt_d = nc.dram_tensor("outt", [D, 1024], F32, kind="ExternalOutput").ap()

    xt_r = xt_d.rearrange("(j p) t -> p j t", p=P)
    xtb_r = xtb_d.rearrange("(j p) t -> p j t", p=P)

    with tile.TileContext(nc) as tc, ExitStack() as ctx, \
         nc.allow_low_precision(reason="fp32r/bf16 intermediates are intended"):
        consts = ctx.enter_context(tc.tile_pool(name="consts", bufs=1))
        ps = ctx.enter_context(tc.tile_pool(name="ps", bufs=1, space="PSUM"))
        rows = ctx.enter_context(tc.tile_pool(name="rows", bufs=1))
        work = ctx.enter_context(tc.tile_pool(name="work", bufs=2))

        onesr_sb = consts.tile([1, P], F32R, tag="onesr")
        onesc_sb = consts.tile([P, 1], F32R, tag="onesc")
        onescb_sb = consts.tile([P, 1], BF16, tag="onescb")
        scale_sb = consts.tile([P, 24], F32, tag="scalein")
        bias_sb = consts.tile([P, 24], F32, tag="biasin")
        g2_sb = consts.tile([P, NC], F32, tag="g2")
        be2_sb = consts.tile([P, NC], F32, tag="be2")
        g2r_sb = consts.tile([1, D], F32R, tag="g2r")
        b1_sb = consts.tile([P, NF], F32, tag="b1")
        ones512_sb = consts.tile([1, TB], F32R, tag="ones512")
        b2s_sb = consts.tile([1, D], F32R, tag="b2s")
        nwqs_sb = consts.tile([1, D], F32R, tag="nwqs")
        nwks_sb = consts.tile([1, D], F32R, tag="nwks")
        nwvs_sb = consts.tile([1, D], F32R, tag="nwvs")
        if use_be1:
            wqbe_sb = consts.tile([1, D], F32R, tag="wqbe")
            wkbe_sb = consts.tile([1, D], F32R, tag="wkbe")
            wvbe_sb = consts.tile([1, D], F32R, tag="wvbe")

        def _early_const_dmas():
            nc.sync.dma_start(onesc_sb[:], onesc_d)
            nc.sync.dma_start(onescb_sb[:], onescb_d)
            nc.sync.dma_start(onesr_sb[:], onesr_d)
            nc.sync.dma_start(nwqs_sb[:], nwqs_d)
            nc.sync.dma_start(nwks_sb[:], nwks_d)
            nc.sync.dma_start(nwvs_sb[:], nwvs_d)
            if use_be1:
                nc.sync.dma_start(wqbe_sb[:], wqbe_d)
                nc.sync.dma_start(wkbe_sb[:], wkbe_d)
                nc.sync.dma_start(wvbe_sb[:], wvbe_d)
                nc.sync.dma_start(ones512_sb[:], ones512_d)

        def _late_const_dmas():
            nc.sync.dma_start(scale_sb[:], scalein_d)
            nc.sync.dma_start(bias_sb[:], biasin_d)
            nc.sync.dma_start(g2r_sb[:], g2r_d)
            for sb, d in ((g2_sb, g2_d), (be2_sb, be2_d)):
                nc.sync.dma_start(sb[:], d.rearrange("(j p) -> p j", p=P))
            nc.sync.dma_start(b1_sb[:], b1_d.rearrange("(j p) -> p j", p=P))
            if not use_be1:
                nc.sync.dma_start(ones512_sb[:], ones512_d)
            nc.sync.dma_start(b2s_sb[:], b2s_d)
            nc.sync.dma_start(masks_sb[:], masks_d.rearrange("o p t -> p o t"))

        def _sums(src_sl, sq_engines, ones, sq_dt):
            """s1/s2 column-sum matmuls; squares cycled over sq_engines.
            `ones`/`sq_dt` must match src dtype class (32-bit vs not)."""
            s1 = ps.tile([1, TB], F32, tag="acc", bufs=2, name="s1")
            s2 = ps.tile([1, TB], F32, tag="acc", bufs=2, name="s2")
            for j in range(NC):
                nc.tensor.matmul(s1[:], ones[:], src_sl[:, j, :],
                                 start=(j == 0), stop=(j == NC - 1))
            for j in range(NC):
                sq = work.tile([P, TB], sq_dt, tag="sq", bufs=3)
                eng = sq_engines[j % len(sq_engines)]
                if eng is nc.scalar:
                    nc.scalar.activation(sq[:], src_sl[:, j, :], AF.Square)
                else:
                    eng.tensor_mul(sq[:], src_sl[:, j, :], src_sl[:, j, :])
                nc.tensor.matmul(s2[:], ones[:], sq[:],
                                 start=(j == 0), stop=(j == NC - 1))
            return s1, s2

        def _muvar(s1, s2):
            mu = rows.tile([1, TB], F32R, tag="mu", bufs=2)
            t = rows.tile([1, TB], F32R, tag="tmp", bufs=2)
            r = rows.tile([1, TB], F32R, tag="r", bufs=2)
            nc.vector.tensor_scalar_mul(mu[:], s1[:], 1.0 / D)
            nc.vector.tensor_mul(t[:], mu[:], mu[:])
            nc.vector.scalar_tensor_tensor(t[:], s2[:], 1.0 / D, t[:],
                                           ALU.mult, ALU.subtract)
            nc.vector.tensor_scalar_add(t[:], t[:], EPS)
            nc.scalar.activation(t[:], t[:], AF.Sqrt)
            nc.vector.reciprocal(r[:], t[:])
            return mu, r, t

        def ln_stats(src_sl):
            """src_sl: [128, NC, TB] slice. Returns (r, mur) rows in SBUF.
            Avoids ACT entirely (it is exp-saturated during attention)."""
            s1, s2 = _sums(src_sl, (nc.vector, nc.gpsimd), onesc_sb, F32R)
            mu, r, _ = _muvar(s1, s2)
            mur = rows.tile([1, TB], F32R, tag="mur", bufs=2)
            nc.vector.tensor_mul(mur[:], mu[:], r[:])
            return r, mur

        def ln1_stats(tb, xt_t):
            """Stats for the folded LN1: returns (mu, bcrs, rcol).

            mu: [1,TB] f32r row (rank-1 correction rhs); bcrs: [P,TB] bf16
            broadcast of 1/sigma (columns); rcol: [P,4] f32 1/sigma along
            partitions for this block's four 128-token s-chunks (V scale).
            """
            sqe = (nc.scalar, nc.vector, nc.gpsimd) if tb == 0 \
                else (nc.scalar, nc.gpsimd)
            s1, s2 = _sums(xt_t[:], sqe, onescb_sb, BF16)
            mu, r, sig = _muvar(s1, s2)
            bcr = ps.tile([P, TB], F32, tag="yt", bufs=2, name="bcs")
            nc.tensor.matmul(bcr[:], onesr_sb[:], r[:], start=True, stop=True)
            bcrs = work.tile([P, TB], BF16, tag="bcrs")
            nc.vector.tensor_copy(bcrs[:], bcr[:])
            rtp = ps.tile([P, TB], F32, tag="yt", bufs=2, name="rtp")
            for si in range(4):
                nc.tensor.matmul(rtp[:, si:si + 1], r[:, si * P:(si + 1) * P],
                                 onesr_sb[:, 0:1], start=True, stop=True)
            rcol = work.tile([P, 4], F32, tag="rcol")
            nc.vector.tensor_copy(rcol[:], rtp[:, 0:4])
            return mu, bcrs, rcol, sig

        def ln_normalize(src_sl, dst_sl, r, mur, g_sb, be_sb, g_row):
            """dst = ((src*g[p])*bc(r) + be[p]) - bc(g[p]*mur)."""
            bcr = ps.tile([P, TB], F32, tag="yt", bufs=2, name="bcs")[:]
            nc.tensor.matmul(bcr, onesr_sb[:], r[:], start=True, stop=True)
            for j in range(NC):
                bc2 = ps.tile([P, TB], F32, tag="acc", bufs=2, name="bc2")
                nc.tensor.matmul(bc2[:], g_row[:, j * P:(j + 1) * P], mur[:],
                                 start=True, stop=True)
                t1 = work.tile([P, TB], F32R, tag="nrm")
                nc.vector.scalar_tensor_tensor(t1[:], src_sl[:, j, :],
                                               g_sb[:, j:j + 1], bcr,
                                               ALU.mult, ALU.mult)
                nc.vector.scalar_tensor_tensor(dst_sl[:, j, :], t1[:],
                                               be_sb[:, j:j + 1], bc2[:],
                                               ALU.add, ALU.subtract)

        # ---------------- Phase 1+2: LN1-folded QKV, software-pipelined -----
        _mark(nc, "ln1")
        es_kqv = ExitStack()
        p_kqv = es_kqv.enter_context(tc.tile_pool(name="p_kqv", bufs=1,
                                                  side="right"))
        kt_sb = p_kqv.tile([P, NC, T], BF16, tag="kt")      # K^T [m, s]
        qt_sb = p_kqv.tile([P, NC, 1024], BF16, tag="qt")   # Q^T [m, t_own]
        v_sb = p_kqv.tile([P, 16, H * 65], BF16, tag="v")   # V_ext [s, (h,65)]
        v_view = v_sb.rearrange("p s (h e) -> p s h e", e=65)
        nc.vector.memset(v_view[:, :, :, 64:65], 1.0)

        es_masks = ExitStack()
        p_masks = es_masks.enter_context(tc.tile_pool(name="p_masks", bufs=1,
                                                      side="right"))
        p_e = es_masks.enter_context(tc.tile_pool(name="p_e", bufs=6,
                                                  side="right"))
        masks_sb = p_masks.tile([P, 4, 2 * TB], BF16, tag="masks")
        # tri chunks (di>0) exp/mask/PV all operate only on the live column
        # sub-range [lo:], so the masked-out columns are never read and need
        # no pre-zeroed tiles.

        es_wqkv = ExitStack()
        p_wqkv = es_wqkv.enter_context(tc.tile_pool(name="p_wqkv", bufs=1,
                                                    side="right"))
        wq_sb = p_wqkv.tile([P, NC, D], BF16, tag="wq")
        wk_sb = p_wqkv.tile([P, NC, D], BF16, tag="wk")
        wv_sb = p_wqkv.tile([P, NC, D], BF16, tag="wv")

        def qkv_for_tb(tb, xt_t, mu, bcrs, rcol, sig):
            """QKV projections straight from raw x^T with LN1 folded in:
            psum accumulates (g1-folded w)@x plus the rank-1 -colsum(w)*mu
            correction; the per-token 1/sigma lands at psum readout (bcrs
            columns for K/Q, rcol per-partition scale for V).
            Accumulators live two-per [P, 2*TB] PSUM tile on the "st" tag
            (idle during phase 1)."""
            tsl = slice(tb * TB, (tb + 1) * TB)
            _mark(nc, "qkv")

            def proj_pair(w_sb, ws_row, wbe_row, dst_sb, mtp):
                acc2 = ps.tile([P, 2 * TB], F32, tag="st", bufs=2, name="ka")
                for half in range(2):
                    mt = 2 * mtp + half
                    msl = slice(mt * P, (mt + 1) * P)
                    seg = acc2[:, half * TB:(half + 1) * TB]
                    for j in range(NC):
                        nc.tensor.matmul(seg, w_sb[:, j, msl], xt_t[:, j, :],
                                         start=(j == 0), stop=False)
                    nc.tensor.matmul(seg, ws_row[:, msl], mu[:],
                                     start=False, stop=(not use_be1))
                    if use_be1:
                        # be-term must survive the *1/sigma readout: rhs=sigma
                        nc.tensor.matmul(seg, wbe_row[:, msl], sig[:],
                                         start=False, stop=True)
                for half in range(2):
                    mt = 2 * mtp + half
                    seg = acc2[:, half * TB:(half + 1) * TB]
                    nc.vector.tensor_mul(dst_sb[:, mt, tsl], seg, bcrs[:])

            for mtp in range(3):
                proj_pair(wk_sb, nwks_sb, wkbe_sb if use_be1 else None,
                          kt_sb, mtp)
            for si in range(4):
                st = tb * 4 + si
                lsl = slice(si * P, (si + 1) * P)
                acc2 = ps.tile([P, 2 * TB], F32, tag="st", bufs=2, name="va")
                for half, fsl, off, w in ((0, slice(0, TB), 0, TB),
                                          (1, slice(TB, D), TB, D - TB)):
                    seg = acc2[:, off:off + w]
                    for j in range(NC):
                        nc.tensor.matmul(seg, xt_t[:, j, lsl],
                                         wv_sb[:, j, fsl],
                                         start=(j == 0), stop=False)
                    nc.tensor.matmul(seg, mu[:, lsl], nwvs_sb[:, fsl],
                                     start=False, stop=(not use_be1))
                    if use_be1:
                        nc.tensor.matmul(seg, sig[:, lsl], wvbe_sb[:, fsl],
                                         start=False, stop=True)
                for half, off, w in ((0, 0, TB), (1, TB, D - TB)):
                    src = acc2[:, off:off + w].rearrange(
                        "p (h e) -> p h e", e=64)
                    h0 = half * 8
                    nc.scalar.activation(
                        v_view[:, st, h0:h0 + w // 64, 0:64], src, AF.Copy,
                        scale=rcol[:, si:si + 1])
            if tb < 2:
                for mtp in range(3):
                    proj_pair(wq_sb, nwqs_sb, wqbe_sb if use_be1 else None,
                              qt_sb, mtp)

        with tc.tile_pool(name="p_xtr", bufs=3) as p_xtr:
            stats = {}
            xts = {}
            for tb in range(NTB):
                tsl = slice(tb * TB, (tb + 1) * TB)
                xt_t = p_xtr.tile([P, NC, TB], BF16, tag="xtr")
                if tb == 0:
                    # tiny consts first (s1's onesc lhsT must not queue
                    # behind the bulk transfers), then the first block split
                    # so the stats tree can start after ~1/3 of it
                    _early_const_dmas()
                    for jj in range(3):
                        nc.sync.dma_start(xt_t[:, 2 * jj:2 * jj + 2, :],
                                          xtb_r[:, 2 * jj:2 * jj + 2, tsl])
                else:
                    nc.sync.dma_start(xt_t[:], xtb_r[:, :, tsl])
                if tb == 0:
                    nc.sync.dma_start(wk_sb[:],
                                      wkt_d.rearrange("(j p) m -> p j m", p=P))
                    nc.sync.dma_start(wv_sb[:],
                                      wvt_d.rearrange("(j p) m -> p j m", p=P))
                    nc.sync.dma_start(wq_sb[:],
                                      wqt_d.rearrange("(j p) m -> p j m", p=P))
                if tb == 1:
                    _late_const_dmas()
                xts[tb] = xt_t
                # emit qkv(tb-1) before stats(tb): the bulk PE work is ready
                # to run, so the in-order PE stream never parks on the
                # square/sum chain of the next block
                if tb > 0:
                    qkv_for_tb(tb - 1, xts[tb - 1][:], *stats[tb - 1])
                stats[tb] = ln1_stats(tb, xt_t)
            qkv_for_tb(NTB - 1, xts[NTB - 1][:], *stats[NTB - 1])
        es_wqkv.close()

        # ---------------- Phase 3: attention (+ per-slot wo/LN2) -----------
        _mark(nc, "attn")
        p_xp = ctx.enter_context(tc.tile_pool(name="p_xp", bufs=1))
        xp_sb = p_xp.tile([P, NC, 1024], F32R, tag="xp")
        p_xn2 = ctx.enter_context(tc.tile_pool(name="p_xn2", bufs=1))
        xn2_sb = p_xn2.tile([P, NC, 1024], F8, tag="xn2")
        es_yt = ExitStack()
        p_yt = es_yt.enter_context(tc.tile_pool(name="p_yt", bufs=2))
        es_wo = ExitStack()
        p_wo = es_wo.enter_context(tc.tile_pool(name="p_wo", bufs=1))
        wo_sb = p_wo.tile([P, NC, D], BF16, tag="wo")
        nc.sync.dma_start(wo_sb[:], wo_d.rearrange("(j p) m -> p j m", p=P))
        xo_sb = p_wo.tile([P, NC, 1024], F32R, tag="xo")
        nc.sync.dma_start(xo_sb[:], xt_r[:, :, 0:1024])

        if True:
            for sl_i in range(2):
                yt_all = p_yt.tile([P, NC, TB], BF16, tag="yt_all")
                qsl = slice(sl_i * TB, (sl_i + 1) * TB)
                chunks = SLOT_CHUNKS[sl_i]
                for mt in range(NC):
                    yt2 = [ps.tile([65, TB], F32, tag="yt", bufs=2,
                                   name=f"yt_{sl_i}_{mt}_{ph}") for ph in range(2)]
                    for ci, ch in enumerate(chunks):
                        sb_idx = (0 if sl_i == 0 else 8) + ci
                        di = ch - 4 * sl_i
                        tri = 0 <= di < 4
                        lo = 128 * di if tri else 0
                        st2 = ps.tile([P, 2 * TB], F32, tag="st", bufs=2)
                        qsub = slice(sl_i * TB + lo, (sl_i + 1) * TB)
                        for ph in range(2):
                            o = ph * 64
                            nc.tensor.matmul(
                                st2[:, ph * TB + lo:(ph + 1) * TB],
                                kt_sb[o:o + 64, mt, ch * P:(ch + 1) * P],
                                qt_sb[o:o + 64, mt, qsub],
                                start=True, stop=True)
                        e_tile = p_e.tile([P, 2 * TB], BF16, tag="e",
                                          name=f"e_{sl_i}_{mt}_{ci}")
                        e_sb = e_tile[:]
                        if lo:
                            ev = e_sb.rearrange("p (two t) -> p two t",
                                                two=2)[:, :, lo:]
                            sv = st2[:].rearrange("p (two t) -> p two t",
                                                  two=2)[:, :, lo:]
                            mv = masks_sb[:, di, :].rearrange(
                                "p (two t) -> p two t", two=2)[:, :, lo:]
                        else:
                            ev, sv = e_sb, st2[:]
                            mv = masks_sb[:, di, :] if tri else None
                        nc.scalar.activation(
                            ev, sv, AF.Exp,
                            bias=bias_sb[:, sb_idx:sb_idx + 1],
                            scale=scale_sb[:, sb_idx:sb_idx + 1])
                        if tri:
                            nc.vector.tensor_mul(ev, ev, mv)
                        for ph in range(2):
                            h = 2 * mt + ph
                            nc.tensor.matmul(
                                yt2[ph][:, lo:], v_sb[:, ch, h * 65:(h + 1) * 65],
                                e_sb.rearrange("p (two t) -> p two t",
                                               two=2)[:, ph, lo:],
                                start=(ci == 0),
                                stop=(ci == len(chunks) - 1))
                    for ph in range(2):
                        o = ph * 64
                        # copy [65,TB] to SBUF immediately: frees the PSUM
                        # bank so the next mt's PV can start during division
                        yt_sb = work.tile([65, TB], F32R, tag="ydiv")
                        nc.vector.tensor_copy(yt_sb[:], yt2[ph][:])
                        rc = rows.tile([1, TB], F32R, tag="r", bufs=2,
                                       name="rc")
                        nc.vector.reciprocal(rc[:], yt_sb[64:65, :])
                        bc = ps.tile([64, TB], F32, tag="acc", bufs=2,
                                     name="abc")
                        nc.tensor.matmul(bc[:], onesr_sb[:, 0:64],
                                         rc[:], start=True, stop=True)
                        dst = yt_all[o:o + 64, mt, :]
                        nc.vector.tensor_mul(dst, yt_sb[0:64, :], bc[:])
                # w_o projection + residual for this slot (fills exp-waits of
                # the other slot)
                _mark(nc, "wo")
                for ct in range(NC):
                    ao = ps.tile([P, TB], F32, tag="acc", bufs=2, name="ao")
                    for mc in range(NC):
                        nc.tensor.matmul(ao[:],
                                         wo_sb[:, mc, ct * P:(ct + 1) * P],
                                         yt_all[:, mc, :],
                                         start=(mc == 0), stop=(mc == NC - 1))
                    nc.vector.tensor_add(xp_sb[:, ct, qsl],
                                         xo_sb[:, ct, qsl], ao[:])
                _mark(nc, "ln2")
                r2, mur2 = ln_stats(xp_sb[:, :, qsl])
                ln_normalize(xp_sb[:, :, qsl], xn2_sb[:, :, qsl],
                             r2, mur2, g2_sb, be2_sb, g2r_sb)

        es_masks.close()
        es_kqv.close()
        es_wo.close()
        es_yt.close()

        # ---------------- Phase 6: MLP ----------------
        _mark(nc, "mlp")
        w1t_r = w1t_d.rearrange("g (j p) two f -> p g j two f", p=P)
        w2t_r = w2t_d.rearrange("(f p) two c -> p f two c", p=P)
        outt_r = outt_d.rearrange("(j p) t -> p j t", p=P)
        with tc.tile_pool(name="p_h1", bufs=1) as p_h1, \
             tc.tile_pool(name="p_wmlp", bufs=3) as p_wmlp, \
             tc.tile_pool(name="p_w2", bufs=1) as p_w2, \
             tc.tile_pool(name="p_out", bufs=4) as p_out:
            h1_sb = p_h1.tile([P, NF, 1024], F8, tag="h1")
            w2_sb = p_w2.tile([P, NF, 2, D], F8, tag="w2full")
            for ft4 in range(NF // 4):
                w1_t = p_wmlp.tile([P, NC, 2, 4 * P], F8, tag="w1")
                nc.sync.dma_start(w1_t[:], w1t_r[:, ft4])
                # stream all of w2 into SBUF once, interleaved with the w1
                # loads so it is resident before (and reused across) both g
                # passes of the second matmul
                nc.sync.dma_start(w2_sb[:, 4 * ft4:4 * ft4 + 4],
                                  w2t_r[:, 4 * ft4:4 * ft4 + 4, :, :])
                for sub in range(4):
                    ft = 4 * ft4 + sub
                    for tb in range(2):
                        tsl = slice(tb * TB, (tb + 1) * TB)
                        hp = ps.tile([P, TB], F32, tag="acc", bufs=2, name="hp")
                        for j in range(NC):
                            nc.tensor.matmul(
                                hp[:],
                                w1_t[:, j, :, sub * P:(sub + 1) * P],
                                xn2_sb[:, j, tsl].unsqueeze(1)
                                .broadcast_to([P, 2, TB]),
                                start=(j == 0), stop=(j == NC - 1),
                                perf_mode=DR)
                        # h1 = S_H1*relu(psum/S_W1 + b1): scale=S_H1/S_W1,
                        # bias = S_H1*b1 (host-prescaled in b1v)
                        nc.scalar.activation(h1_sb[:, ft, tsl], hp[:], AF.Relu,
                                             bias=b1_sb[:, ft:ft + 1],
                                             scale=S_H1 / S_W1)
            for g in range(2):
                o2s = {}
                for ci, (t, bu) in enumerate((("acc", 2), ("yt", 2))):
                    for tb in range(2):
                        o2s[(ci, tb)] = ps.tile([P, TB], F32, tag=t, bufs=bu,
                                                name=f"o2_{g}_{ci}_{tb}")
                stp = ps.tile([P, 2 * TB], F32, tag="st", bufs=2,
                              name=f"o2st_{g}")
                o2s[(2, 0)] = stp[:, 0:TB]
                o2s[(2, 1)] = stp[:, TB:2 * TB]
                # seed each accumulator with (b2/C_MLP) x ones
                for ci in range(3):
                    ct = g * 3 + ci
                    for tb in range(2):
                        nc.tensor.matmul(o2s[(ci, tb)][:],
                                         b2s_sb[:, ct * P:(ct + 1) * P],
                                         ones512_sb[:],
                                         start=True, stop=False)
                for ft in range(NF):
                    for ci in range(3):
                        ct = g * 3 + ci
                        for tb in range(2):
                            nc.tensor.matmul(
                                o2s[(ci, tb)][:],
                                w2_sb[:, ft, :, ct * P:(ct + 1) * P],
                                h1_sb[:, ft, tb * TB:(tb + 1) * TB]
                                .unsqueeze(1).broadcast_to([P, 2, TB]),
                                start=False, stop=(ft == NF - 1),
                                perf_mode=DR)
                for tb in range(2):
                    tsl = slice(tb * TB, (tb + 1) * TB)
                    for ci in range(3):
                        ct = g * 3 + ci
                        ot = p_out.tile([P, TB], F32, tag="ot",
                                        name=f"ot_{g}_{tb}_{ci}")
                        nc.vector.scalar_tensor_tensor(
                            ot[:], o2s[(ci, tb)][:], C_MLP,
                            xp_sb[:, ct, tsl], ALU.mult, ALU.add)
                        nc.sync.dma_start(outt_r[:, ct, tsl], ot[:])

    nc.compile()
    return nc


def _hilo(w, f8):
    """[..., n] -> [..., 2, n] fp8 (hi, residual-lo) planes."""
    hi = w.astype(f8)
    lo = (w - hi.astype(np.float32)).astype(f8)
    return np.ascontiguousarray(np.stack([hi, lo], axis=-2))


def _host_inputs(X, w_q, w_k, w_v, w_o, W1, b1, W2, b2, g1, be1, g2, be2):
    """Build the 8 per-core input dicts."""
    f32 = np.float32
    import ml_dtypes as _mld
    _f8 = _mld.float8_e4m3
    _bf = _mld.bfloat16
    g1v = np.asarray(g1, f32)
    be1v = np.asarray(be1, f32)
    # LN1 fold: g1 into the QKV weight columns; mean correction rows are the
    # negated column sums; optional be1 rows handle a nonzero LN1 shift
    wqg = np.asarray(w_q, f32).reshape(D, D) * g1v[None, :]
    wkg = np.asarray(w_k, f32).reshape(D, D) * g1v[None, :]
    wvg = np.asarray(w_v, f32).reshape(D, D) * g1v[None, :]
    wqt = np.ascontiguousarray(wqg.T.astype(_bf))
    wkt = np.ascontiguousarray(wkg.T.astype(_bf))
    wvt = np.ascontiguousarray(wvg.T.astype(_bf))
    nwqs = np.ascontiguousarray(-wqg.sum(axis=1).reshape(1, D))
    nwks = np.ascontiguousarray(-wkg.sum(axis=1).reshape(1, D))
    nwvs = np.ascontiguousarray(-wvg.sum(axis=1).reshape(1, D))
    use_be1 = bool(np.any(be1v))
    wo = np.ascontiguousarray(np.asarray(w_o, f32).astype(_bf))
    w1t = None  # bf16, set below
    w2t = None  # bf16, set below
    onesr = np.ones((1, P), f32)
    onesc = np.ones((P, 1), f32)
    onescb = np.ones((P, 1), _bf)
    onesv = None  # set below after bf16 import
    # 4 canonical self-diagonal masks: mask[k][s, t] = (128k + s <= t)
    import ml_dtypes
    bf16 = ml_dtypes.bfloat16
    masks = np.zeros((4, P, 2 * TB), bf16)
    ar_s = np.arange(P)[:, None]
    ar_t = np.arange(TB)[None, :]
    for k in range(4):
        m = (128 * k + ar_s <= ar_t).astype(bf16)
        masks[k, :, 0:TB] = m
        masks[k, :, TB:2 * TB] = m
    w1t = _hilo(np.asarray(W1, f32).T * S_W1, _f8)   # [D, 2, DFF]
    w1t = np.ascontiguousarray(
        w1t.reshape(D, 2, NF // 4, 4 * P).transpose(2, 0, 1, 3))
    w2t = _hilo(np.asarray(W2, f32).T * S_W2, _f8)

    # per-role exp scale/bias: 24 = 8 (slot0) + 16 (slot1) chunk positions
    sc = {}
    bi = {}
    for role in range(2):
        order = ROLE_ORDER[role]
        s = np.full((24,), 0.125, f32)
        b = np.zeros((24,), f32)
        for sl_i in range(2):
            own_blk = order[sl_i]
            for ci, ch in enumerate(SLOT_CHUNKS[sl_i]):
                idx = (0 if sl_i == 0 else 8) + ci
                pos = ch // 4           # permuted 512-block of this s-chunk
                blk = order[pos]
                if pos == sl_i or blk < own_blk:
                    pass                # diagonal (tri-masked) or past: live
                else:
                    s[idx] = 0.0        # future: dead
                    b[idx] = DEAD
        sc[role] = np.broadcast_to(s, (P, 24)).copy()
        bi[role] = np.broadcast_to(b, (P, 24)).copy()

    g2r = np.asarray(g2, f32).reshape(1, D)
    shared = dict(wqt=wqt, wkt=wkt, wvt=wvt, wo=wo, w1t=w1t, w2t=w2t,
                  g2r=g2r, nwqs=nwqs, nwks=nwks, nwvs=nwvs,
                  onesr=onesr, onesc=onesc, onescb=onescb, masks=masks,
                  g2v=np.asarray(g2, f32), be2v=np.asarray(be2, f32),
                  b1v=np.asarray(b1, f32) * S_H1,
                  b2s=np.asarray(b2, f32).reshape(1, D) * (S_H1 * S_W2),
                  ones512=np.ones((1, TB), f32))
    if use_be1:
        shared["wqbe"] = (np.asarray(w_q, f32).reshape(D, D) @ be1v
                          ).reshape(1, D).astype(f32)
        shared["wkbe"] = (np.asarray(w_k, f32).reshape(D, D) @ be1v
                          ).reshape(1, D).astype(f32)
        shared["wvbe"] = (np.asarray(w_v, f32).reshape(D, D) @ be1v
                          ).reshape(1, D).astype(f32)

    in_maps = []
    for core in range(8):
        role, b_idx = core // 4, core % 4
        order = ROLE_ORDER[role]
        xb = np.asarray(X[b_idx], f32)          # [T, D]
        xperm = np.concatenate([xb[o * TB:(o + 1) * TB] for o in order], axis=0)
        xt = np.ascontiguousarray(xperm.T)      # [D, T]
        m = dict(shared)
        m["xt"] = xt
        m["xtb"] = np.ascontiguousarray(xt.astype(_bf))
        m["scalein"] = sc[role]
        m["biasin"] = bi[role]
        in_maps.append(m)
    return in_maps


def _assemble(results, dtype):
    out = np.empty((B, T, D), dtype)
    for core in range(8):
        role, b_idx = core // 4, core % 4
        order = ROLE_ORDER[role]
        ot = results[core]["outt"]              # [D, 1024]
        for sl_i in range(2):
            blk = order[sl_i]
            out[b_idx, blk * TB:(blk + 1) * TB] = \
                ot[:, sl_i * TB:(sl_i + 1) * TB].T
    return out


def kernel(X, w_q, w_k, w_v, w_o, W1, b1, W2, b2, g1, be1, g2, be2,
           _want_results=False, _trace=False):
    use_be1 = bool(np.any(np.asarray(be1)))
    key = ("nc", use_be1)
    if key not in _cached:
        _cached[key] = _build_nc(use_be1=use_be1)
        _cached["nc"] = _cached[key]
    nc = _cached[key]
    in_maps = _host_inputs(X, w_q, w_k, w_v, w_o, W1, b1, W2, b2,
                           g1, be1, g2, be2)
    res = run_bass_kernel_spmd(nc, in_maps, core_ids=list(range(8)),
                               trace=_trace)
    out = _assemble(res.results, np.asarray(X).dtype)
    if _want_results:
        return out, res
    return out

